# revision 1
# baseline (speedup 1.0000x reference)
"""Trainium2 Bass kernel for nn_Block_59433757442280 (spiking-NN local-attention block).

Sharding: data-parallel over B=8 (one batch element per NeuronCore), no collectives.
On-device layout: activations transposed [C, tok] with tok = t*1024 + n.
All GEMMs in bf16 (validated: reproduces the fp32 reference exactly on these inputs,
the LIF threshold margins downstream are structural); LIF membrane state in bf16,
softmax in fp32/bf16. The local-attention mask is folded into the sim matmul via
16 extra one-hot contraction rows.
"""

import sys

for _p in ("/opt/trn_rl_repo",):
    if _p not in sys.path:
        sys.path.insert(0, _p)

import numpy as np
import ml_dtypes

import concourse.bass as bass
import concourse.tile as tile
from concourse import mybir, bacc
from concourse.bass_utils import run_bass_kernel_spmd

F32 = mybir.dt.float32
BF16 = mybir.dt.bfloat16
AF = mybir.ActivationFunctionType
ALU = mybir.AluOpType
BF = ml_dtypes.bfloat16

# problem constants
T, B, NSEQ, C, HD = 4, 8, 1024, 768, 3072
NH, DH, W = 8, 96, 8
TOK = T * NSEQ                      # 4096 tokens per core
SCALE = float(DH) ** -0.5
NEG = -30000.0                      # mask offset (exp(scale*NEG) == 0.0 in fp32)
NCH = 256                           # phase-A/B chunk size along n
NCHUNK = NSEQ // NCH                # 4 chunks
CI6 = C // 128                      # 6 contraction tiles of 128
M24 = HD // 128                     # 24 f1 output tiles
VTH2 = 2.0                          # 2*vth for qkv/proj/mlp LIF
VTH2A = 1.0                         # 2*vth for attn lif (vth=0.5)


def _lif_head(nc, pools, psum_ap_of_t, bias_ap, dst_of_t, vth2, nt=T):
    """Emit LIF over t for one tile group.

    psum_ap_of_t(t) -> [P, n] fp32 PSUM AP of the GEMM output at step t
    bias_ap        -> [P, 1] f32 SBUF AP (per-partition bias) or None
    dst_of_t(t)    -> [P, n] bf16 SBUF AP to write spikes into
    u state: u_{t+1} = u_t * (0.5*[u_t < vth2]) + y_{t+1};  s_t = [u_t >= vth2]
    """
    upool, gpool = pools
    u_prev = None
    for t in range(nt):
        y = psum_ap_of_t(t)
        p, n = y.shape[0], y.shape[-1]
        if u_prev is None:
            u = upool.tile([p, n], BF16, name="u", tag="u")
            if bias_ap is not None:
                nc.scalar.activation(u[:], y, AF.Identity, bias=bias_ap)
            else:
                nc.vector.tensor_copy(u[:], y)
        else:
            g = gpool.tile([p, n], BF16, name="g", tag="g")
            nc.vector.tensor_scalar(g[:], u_prev[:], vth2, 0.5, ALU.is_lt, ALU.mult)
            ug = gpool.tile([p, n], BF16, name="ug", tag="ug")
            nc.gpsimd.tensor_tensor(ug[:], u_prev[:], g[:], ALU.mult)
            if bias_ap is not None:
                yb = gpool.tile([p, n], BF16, name="yb", tag="yb")
                nc.scalar.activation(yb[:], y, AF.Identity, bias=bias_ap)
                u = upool.tile([p, n], BF16, name="u", tag="u")
                nc.vector.tensor_tensor(u[:], ug[:], yb[:], ALU.add)
            else:
                u = upool.tile([p, n], BF16, name="u", tag="u")
                nc.vector.tensor_tensor(u[:], ug[:], y, ALU.add)
        nc.vector.tensor_scalar(dst_of_t(t), u[:], vth2, None, ALU.is_ge)
        u_prev = u


def build_nc(debug=False):
    nc = bacc.Bacc(None, target_bir_lowering=False, debug=False)

    # ---- DRAM declarations (per core) ----
    xT_bf = nc.dram_tensor("xT_bf", [CI6, 128, TOK], BF16, kind="ExternalInput")
    xT_f32 = nc.dram_tensor("xT_f32", [CI6, 128, TOK], F32, kind="ExternalInput")
    wq_d = nc.dram_tensor("wq", [128, CI6 * C], BF16, kind="ExternalInput")
    wk_d = nc.dram_tensor("wk", [128, CI6 * C], BF16, kind="ExternalInput")
    wv_d = nc.dram_tensor("wv", [128, CI6 * C], BF16, kind="ExternalInput")
    wp_d = nc.dram_tensor("wp", [128, CI6 * C], BF16, kind="ExternalInput")
    w1_d = nc.dram_tensor("w1", [128, CI6 * HD], BF16, kind="ExternalInput")
    w2_d = nc.dram_tensor("w2", [128, M24 * C], BF16, kind="ExternalInput")
    bq_d = nc.dram_tensor("bq", [128, CI6], F32, kind="ExternalInput")
    bk_d = nc.dram_tensor("bk", [128, CI6], F32, kind="ExternalInput")
    bvf_d = nc.dram_tensor("bvf", [128, C], BF16, kind="ExternalInput")
    bp_d = nc.dram_tensor("bp", [128, CI6], F32, kind="ExternalInput")
    b1_d = nc.dram_tensor("b1", [128, M24], F32, kind="ExternalInput")
    b2_d = nc.dram_tensor("b2", [128, CI6], F32, kind="ExternalInput")
    qext_d = nc.dram_tensor("qext_pat", [16, NCH * T], BF16, kind="ExternalInput")
    kext_d = nc.dram_tensor("kext_pat", [16, NCH * T], BF16, kind="ExternalInput")
    khp_d = nc.dram_tensor("khalo_pat", [16, T * W], BF16, kind="ExternalInput")
    khf_d = nc.dram_tensor("khalo_first", [16, T * W], BF16, kind="ExternalInput")
    id_d = nc.dram_tensor("ident", [128, 128], BF16, kind="ExternalInput")

    opT = nc.dram_tensor("opT", [CI6, 128, TOK], BF16,
                         kind="ExternalOutput" if debug else "Internal")
    outT = nc.dram_tensor("outT", [CI6, 128, TOK], F32, kind="ExternalOutput")
    if debug:
        dbg_q = nc.dram_tensor("dbg_q", [NH, DH, TOK], BF16, kind="ExternalOutput")
        dbg_k = nc.dram_tensor("dbg_k", [NH, DH, TOK], BF16, kind="ExternalOutput")
        dbg_v = nc.dram_tensor("dbg_v", [TOK, C], BF16, kind="ExternalOutput")
        dbg_oa = nc.dram_tensor("dbg_oa", [NH, DH, TOK], BF16, kind="ExternalOutput")
        dbg_h = nc.dram_tensor("dbg_h", [M24, 128, TOK], BF16, kind="ExternalOutput")

    def tok3(dram_i, c):
        """chunk AP [128, T, NCH] of dram tensor slice i at chunk c"""
        return dram_i.rearrange("p (t n) -> p t n", t=T)[:, :, c * NCH:(c + 1) * NCH]

    with tile.TileContext(nc) as tc:
        from contextlib import ExitStack
        with ExitStack() as top:
            # ======================= PHASE A =======================
            pa = top.enter_context(ExitStack())
            cpool = pa.enter_context(tc.tile_pool(name="const", bufs=1))
            # persistent attention tiles
            perspool = pa.enter_context(tc.tile_pool(name="pers", bufs=1))

            ident = cpool.tile([128, 128], BF16, name="ident", tag="ident")
            nc.gpsimd.dma_start(ident[:], id_d[:])
            bq_sb = cpool.tile([128, CI6], F32, name="bq", tag="bq")
            nc.gpsimd.dma_start(bq_sb[:], bq_d[:])
            bk_sb = cpool.tile([128, CI6], F32, name="bk", tag="bk")
            nc.gpsimd.dma_start(bk_sb[:], bk_d[:])
            bvf_sb = cpool.tile([128, C], BF16, name="bvf", tag="bvf")
            nc.gpsimd.dma_start(bvf_sb[:], bvf_d[:])
            bp_sb = cpool.tile([128, CI6], F32, name="bp", tag="bp")
            nc.gpsimd.dma_start(bp_sb[:], bp_d[:])

            q_ext = [perspool.tile([112, T * NCH], BF16, name=f"qx{h}", tag=f"qx{h}") for h in range(NH)]
            k_ext = [perspool.tile([112, T * NCH], BF16, name=f"kx{h}", tag=f"kx{h}") for h in range(NH)]
            kh_cur = [perspool.tile([112, T, W], BF16, name=f"khc{h}", tag=f"khc{h}") for h in range(NH)]
            kh_prev = [perspool.tile([112, T, W], BF16, name=f"khp{h}", tag=f"khp{h}") for h in range(NH)]
            kh_first = [perspool.tile([112, T, W], BF16, name=f"khf{h}", tag=f"khf{h}") for h in range(NH)]
            vt_t = [[perspool.tile([128, C], BF16, name=f"vt{t}_{qh}", tag=f"vt{t}_{qh}") for qh in range(2)]
                    for t in range(T)]
            vh_cur = [perspool.tile([W, C], BF16, name=f"vhc{t}", tag=f"vhc{t}") for t in range(T)]
            vh_prev = [perspool.tile([W, C], BF16, name=f"vhp{t}", tag=f"vhp{t}") for t in range(T)]
            oa = [perspool.tile([DH, T * NCH], BF16, name=f"oa{h}", tag=f"oa{h}") for h in range(NH)]

            # init ext rows / halos
            for h in range(NH):
                nc.gpsimd.dma_start(q_ext[h][96:112, :], qext_d[:])
                nc.gpsimd.dma_start(k_ext[h][96:112, :], kext_d[:])
                nc.gpsimd.dma_start(
                    kh_cur[h][96:112, :, :], khp_d.rearrange("g (t w) -> g t w", t=T))
                nc.gpsimd.dma_start(
                    kh_prev[h][96:112, :, :], khp_d.rearrange("g (t w) -> g t w", t=T))
                nc.gpsimd.dma_start(
                    kh_first[h][96:112, :, :], khf_d.rearrange("g (t w) -> g t w", t=T))
                nc.vector.memset(kh_prev[h][0:96, :, :], 0.0)
                nc.vector.memset(kh_first[h][0:96, :, :], 0.0)
            for t in range(T):
                nc.vector.memset(vh_prev[t][:], 0.0)

            if True:
                wpoolA = pa.enter_context(tc.tile_pool(name="wA", bufs=1))
                xpool = pa.enter_context(tc.tile_pool(name="xA", bufs=2))
                upool = pa.enter_context(tc.tile_pool(name="uA", bufs=3))
                gpool = pa.enter_context(tc.tile_pool(name="gA", bufs=2))
                apool = pa.enter_context(tc.tile_pool(name="attn", bufs=3))
                oppool = pa.enter_context(tc.tile_pool(name="op", bufs=2))
                spkpool = pa.enter_context(tc.tile_pool(name="spk", bufs=1))
                qkv_ps = pa.enter_context(
                    tc.tile_pool(name="qkvps", bufs=4, space="PSUM"))
                sm_ps = pa.enter_context(
                    tc.tile_pool(name="smps", bufs=4, space="PSUM"))

                wq_sb = wpoolA.tile([128, CI6 * C], BF16, name="wq", tag="wq")
                nc.scalar.dma_start(wq_sb[:], wq_d[:])
                wk_sb = wpoolA.tile([128, CI6 * C], BF16, name="wk", tag="wk")
                nc.gpsimd.dma_start(wk_sb[:], wk_d[:])
                wv_sb = wpoolA.tile([128, CI6 * C], BF16, name="wv", tag="wv")
                nc.sync.dma_start(wv_sb[:], wv_d[:])
                wp_sb = wpoolA.tile([128, CI6 * C], BF16, name="wp", tag="wp")
                nc.scalar.dma_start(wp_sb[:], wp_d[:])

                for c in range(NCHUNK):
                    first_chunk = (c == 0)
                    # ---- load x chunk ----
                    xc = []
                    for i in range(CI6):
                        xi = xpool.tile([128, T, NCH], BF16, name=f"xc{i}", tag=f"xc{i}")
                        nc.sync.dma_start(xi[:], tok3(xT_bf[i], c))
                        xc.append(xi)

                    # ---- q, k GEMM (M=128) + LIF + repack -> q_ext/k_ext rows 0:96 ----
                    for w_sb, b_sb, ext, snm in ((wq_sb, bq_sb, q_ext, "qs"),
                                                 (wk_sb, bk_sb, k_ext, "ks")):
                        s_tmp = []
                        for i in range(CI6):
                            psh = [qkv_ps.tile([128, 2, NCH], F32, name="qkvps",
                                               tag="qkvps") for _ in range(2)]
                            for ci in range(CI6):
                                lhsT = w_sb[:, ci * C + i * 128:ci * C + (i + 1) * 128]
                                for hf in range(2):
                                    nc.tensor.matmul(
                                        psh[hf][:], lhsT,
                                        xc[ci][:, 2 * hf:2 * hf + 2, :],
                                        start=(ci == 0), stop=(ci == CI6 - 1))
                            st = spkpool.tile([128, T * NCH], BF16, name=f"{snm}{i}",
                                              tag=f"{snm}{i}")
                            _lif_head(
                                nc, (upool, gpool),
                                lambda t, psh=psh: psh[t // 2][:, t % 2, :],
                                b_sb[:, i:i + 1],
                                lambda t, st=st: st[:, t * NCH:(t + 1) * NCH],
                                VTH2)
                            s_tmp.append(st)
                        for h in range(NH):
                            cst = h * DH
                            i0, r0 = cst // 128, cst % 128
                            l0 = min(128 - r0, DH)
                            nc.sync.dma_start(ext[h][0:l0, :], s_tmp[i0][r0:r0 + l0, :])
                            if l0 < DH:
                                nc.sync.dma_start(ext[h][l0:DH, :],
                                                  s_tmp[i0 + 1][0:DH - l0, :])

                    # ---- v GEMM (x-stationary -> v.T layout) + LIF ----
                    for qh in range(2):
                        psv_of_t = []
                        for t in range(T):
                            psv = [qkv_ps.tile([128, 384], F32, name="qkvps",
                                               tag="qkvps") for _ in range(2)]
                            for ci in range(CI6):
                                stat = xc[ci][:, t, qh * 128:(qh + 1) * 128]
                                nc.tensor.matmul(psv[0][:], stat,
                                                 wv_sb[:, ci * C:ci * C + 384],
                                                 start=(ci == 0), stop=(ci == CI6 - 1))
                                nc.tensor.matmul(psv[1][:], stat,
                                                 wv_sb[:, ci * C + 384:(ci + 1) * C],
                                                 start=(ci == 0), stop=(ci == CI6 - 1))
                            psv_of_t.append(psv)
                        # LIF over t in v.T layout with full-width bias
                        u_prev = None
                        for t in range(T):
                            y = psv_of_t[t]
                            if u_prev is None:
                                u = upool.tile([128, C], BF16, name="uv", tag="uv")
                                nc.vector.tensor_tensor(u[:, 0:384], y[0][:],
                                                        bvf_sb[:, 0:384], ALU.add)
                                nc.vector.tensor_tensor(u[:, 384:C], y[1][:],
                                                        bvf_sb[:, 384:C], ALU.add)
                            else:
                                g = gpool.tile([128, C], BF16, name="gv", tag="gv")
                                nc.vector.tensor_scalar(g[:], u_prev[:], VTH2, 0.5,
                                                        ALU.is_lt, ALU.mult)
                                ug = gpool.tile([128, C], BF16, name="ugv", tag="ugv")
                                nc.gpsimd.tensor_tensor(ug[:], u_prev[:], g[:], ALU.mult)
                                ub = gpool.tile([128, C], BF16, name="ubv", tag="ubv")
                                nc.gpsimd.tensor_tensor(ub[:], ug[:], bvf_sb[:], ALU.add)
                                u = upool.tile([128, C], BF16, name="uv", tag="uv")
                                nc.vector.tensor_tensor(u[:, 0:384], ub[:, 0:384],
                                                        y[0][:], ALU.add)
                                nc.vector.tensor_tensor(u[:, 384:C], ub[:, 384:C],
                                                        y[1][:], ALU.add)
                            nc.vector.tensor_scalar(vt_t[t][qh][:], u[:], VTH2, None,
                                                    ALU.is_ge)
                            u_prev = u

                    # halo captures needed within this chunk (qb=1 halos)
                    for h in range(NH):
                        nc.vector.tensor_copy(
                            kh_cur[h][0:96, :, :],
                            k_ext[h][0:96, :].rearrange("p (t n) -> p t n", t=T)
                            [:, :, 120:128])
                    for t in range(T):
                        nc.sync.dma_start(vh_cur[t][:], vt_t[t][0][120:128, :])

                    # ---- attention + attn-LIF -> oa ----
                    for h in range(NH):
                        u_o = None
                        for t in range(T):
                            o_ps = sm_ps.tile([DH, NCH], F32, name="attnsm", tag="attnsm")
                            for qb in range(2):
                                qc = t * NCH + qb * 128
                                sim = sm_ps.tile([128, 136], F32, name="attnsm", tag="attnsm")
                                nc.tensor.matmul(
                                    sim[:, 0:128], q_ext[h][0:112, qc:qc + 128],
                                    k_ext[h][0:112, qc:qc + 128], start=True, stop=True)
                                halo = (kh_first[h] if (first_chunk and qb == 0)
                                        else kh_prev[h] if qb == 0 else kh_cur[h])
                                nc.tensor.matmul(
                                    sim[:, 128:136], q_ext[h][0:112, qc:qc + 128],
                                    halo[0:112, t, :], start=True, stop=True)
                                attn = apool.tile([128, 136], BF16, name="attn", tag="attn")
                                rs = apool.tile([128, 1], F32, name="rs", tag="rs")
                                nc.scalar.activation(attn[:], sim[:], AF.Exp,
                                                     scale=SCALE, accum_out=rs[:])
                                rc = apool.tile([128, 1], F32, name="rc", tag="rc")
                                nc.vector.reciprocal(rc[:], rs[:])
                                attn_n = apool.tile([128, 136], BF16, name="attnn", tag="attnn")
                                nc.vector.tensor_scalar(attn_n[:], attn[:], rc[:], None,
                                                        ALU.mult)
                                tpm = sm_ps.tile([128, 128], BF16, name="attnsm", tag="attnsm")
                                nc.tensor.transpose(tpm[:], attn_n[:, 0:128], ident[:])
                                tph = sm_ps.tile([8, 128], BF16, name="attnsm", tag="attnsm")
                                nc.tensor.transpose(tph[:], attn_n[:, 128:136], ident[:])
                                am = apool.tile([128, 128], BF16, name="am", tag="am")
                                nc.scalar.copy(am[:], tpm[:])
                                ah = apool.tile([8, 128], BF16, name="ah", tag="ah")
                                nc.vector.tensor_copy(ah[:], tph[:])
                                vmain = vt_t[t][qb][:, h * DH:(h + 1) * DH]
                                vhalo = (vh_prev[t] if qb == 0 else vh_cur[t])
                                nc.tensor.matmul(o_ps[:, qb * 128:(qb + 1) * 128],
                                                 vmain, am[:], start=True, stop=False)
                                nc.tensor.matmul(o_ps[:, qb * 128:(qb + 1) * 128],
                                                 vhalo[:, h * DH:(h + 1) * DH], ah[:],
                                                 start=False, stop=True)
                            # attn-LIF step t (vth=0.5 -> threshold 1.0 on u)
                            if u_o is None:
                                u = upool.tile([DH, NCH], BF16, name="uo", tag="uo")
                                nc.scalar.copy(u[:], o_ps[:])
                            else:
                                g = gpool.tile([DH, NCH], BF16, name="go", tag="go")
                                nc.vector.tensor_scalar(g[:], u_o[:], VTH2A, 0.5,
                                                        ALU.is_lt, ALU.mult)
                                ug = gpool.tile([DH, NCH], BF16, name="ugo", tag="ugo")
                                nc.gpsimd.tensor_tensor(ug[:], u_o[:], g[:], ALU.mult)
                                u = upool.tile([DH, NCH], BF16, name="uo", tag="uo")
                                nc.vector.tensor_tensor(u[:], ug[:], o_ps[:], ALU.add)
                            nc.vector.tensor_scalar(
                                oa[h][:, t * NCH:(t + 1) * NCH], u[:], VTH2A, None,
                                ALU.is_ge)
                            u_o = u

                    # halo captures for the NEXT chunk (emit after attention reads)
                    for h in range(NH):
                        nc.vector.tensor_copy(
                            kh_prev[h][0:96, :, :],
                            k_ext[h][0:96, :].rearrange("p (t n) -> p t n", t=T)
                            [:, :, NCH - 8:NCH])
                    for t in range(T):
                        nc.sync.dma_start(vh_prev[t][:], vt_t[t][1][120:128, :])

                    # ---- oa repack to 128-tiles, proj GEMM (K=128) + LIF ----
                    oa128 = []
                    for i in range(CI6):
                        ot = spkpool.tile([128, T * NCH], BF16, name=f"oa128_{i}",
                                          tag=f"oa128_{i}")
                        oa128.append(ot)
                    for h in range(NH):
                        cst = h * DH
                        i0, r0 = cst // 128, cst % 128
                        l0 = min(128 - r0, DH)
                        nc.sync.dma_start(oa128[i0][r0:r0 + l0, :], oa[h][0:l0, :])
                        if l0 < DH:
                            nc.sync.dma_start(oa128[i0 + 1][0:DH - l0, :],
                                              oa[h][l0:DH, :])
                    for i in range(CI6):
                        psp = [qkv_ps.tile([128, 2, NCH], F32, name="qkvps",
                                           tag="qkvps") for _ in range(2)]
                        for ci in range(CI6):
                            lhsT = wp_sb[:, ci * C + i * 128:ci * C + (i + 1) * 128]
                            rhs3 = oa128[ci][:, :].rearrange("p (t n) -> p t n", t=T)
                            for hf in range(2):
                                nc.tensor.matmul(
                                    psp[hf][:], lhsT,
                                    rhs3[:, 2 * hf:2 * hf + 2, :],
                                    start=(ci == 0), stop=(ci == CI6 - 1))
                        opc = oppool.tile([128, T, NCH], BF16, name="opc", tag="opc")
                        _lif_head(nc, (upool, gpool),
                                  lambda t, psp=psp: psp[t // 2][:, t % 2, :],
                                  bp_sb[:, i:i + 1],
                                  lambda t, opc=opc: opc[:, t, :],
                                  VTH2)
                        nc.sync.dma_start(tok3(opT[i], c), opc[:])

                    if debug:
                        for h in range(NH):
                            nc.sync.dma_start(
                                dbg_q.rearrange("h d (t n) -> h d t n", t=T)
                                [h][:, :, c * NCH:(c + 1) * NCH],
                                q_ext[h][0:96, :].rearrange("p (t n) -> p t n", t=T))
                            nc.sync.dma_start(
                                dbg_k.rearrange("h d (t n) -> h d t n", t=T)
                                [h][:, :, c * NCH:(c + 1) * NCH],
                                k_ext[h][0:96, :].rearrange("p (t n) -> p t n", t=T))
                            nc.sync.dma_start(
                                dbg_oa.rearrange("h d (t n) -> h d t n", t=T)
                                [h][:, :, c * NCH:(c + 1) * NCH],
                                oa[h][:, :].rearrange("p (t n) -> p t n", t=T))
                        for t in range(T):
                            for qh in range(2):
                                nc.sync.dma_start(
                                    dbg_v[t * NSEQ + c * NCH + qh * 128:
                                          t * NSEQ + c * NCH + (qh + 1) * 128, :],
                                    vt_t[t][qh][:])

            pa.close()
            # ======================= PHASE B =======================
            with ExitStack() as pb:
                wpoolB = pb.enter_context(tc.tile_pool(name="wB", bufs=1))
                xbpool = pb.enter_context(tc.tile_pool(name="xB", bufs=1))
                hpool = pb.enter_context(tc.tile_pool(name="hB", bufs=1))
                ubpool = pb.enter_context(tc.tile_pool(name="uB", bufs=3))
                gbpool = pb.enter_context(tc.tile_pool(name="gB", bufs=2))
                obpool = pb.enter_context(tc.tile_pool(name="oB", bufs=2))
                b_ps = pb.enter_context(tc.tile_pool(name="bps", bufs=4, space="PSUM"))

                w1_sb = wpoolB.tile([128, CI6 * HD], BF16, name="w1", tag="w1")
                nc.scalar.dma_start(w1_sb[:, 0:CI6 * HD // 2], w1_d[:, 0:CI6 * HD // 2])
                nc.sync.dma_start(w1_sb[:, CI6 * HD // 2:], w1_d[:, CI6 * HD // 2:])
                w2_sb = wpoolB.tile([128, M24 * C], BF16, name="w2", tag="w2")
                nc.gpsimd.dma_start(w2_sb[:, 0:M24 * C // 2], w2_d[:, 0:M24 * C // 2])
                nc.gpsimd.dma_start(w2_sb[:, M24 * C // 2:], w2_d[:, M24 * C // 2:])
                b1_sb = wpoolB.tile([128, M24], F32, name="b1", tag="b1")
                nc.sync.dma_start(b1_sb[:], b1_d[:])
                b2_sb = wpoolB.tile([128, CI6], F32, name="b2", tag="b2")
                nc.sync.dma_start(b2_sb[:], b2_d[:])

                for c in range(NCHUNK):
                    xb, opb, x2 = [], [], []
                    for i in range(CI6):
                        xi = xbpool.tile([128, T, NCH], BF16, name=f"xb{i}", tag=f"xb{i}")
                        nc.sync.dma_start(xi[:], tok3(xT_bf[i], c))
                        xb.append(xi)
                        oi = xbpool.tile([128, T, NCH], BF16, name=f"ob{i}", tag=f"ob{i}")
                        nc.sync.dma_start(oi[:], tok3(opT[i], c))
                        opb.append(oi)
                        x2i = xbpool.tile([128, T, NCH], BF16, name=f"x2{i}", tag=f"x2{i}")
                        nc.gpsimd.tensor_tensor(x2i[:], xi[:], oi[:], ALU.add)
                        x2.append(x2i)

                    h_tiles = []
                    for m in range(M24):
                        ps1 = b_ps.tile([128, T, NCH], F32, name="bps", tag="bps")
                        for ci in range(CI6):
                            lhsT = w1_sb[:, ci * HD + m * 128:ci * HD + (m + 1) * 128]
                            for hf in range(2):
                                nc.tensor.matmul(
                                    ps1[:, 2 * hf:2 * hf + 2, :], lhsT,
                                    x2[ci][:, 2 * hf:2 * hf + 2, :],
                                    start=(ci == 0), stop=(ci == CI6 - 1))
                        hm = hpool.tile([128, T, NCH], BF16, name=f"h{m}", tag=f"h{m}")
                        _lif_head(nc, (ubpool, gbpool),
                                  lambda t, ps1=ps1: ps1[:, t, :],
                                  b1_sb[:, m:m + 1],
                                  lambda t, hm=hm: hm[:, t, :],
                                  VTH2)
                        h_tiles.append(hm)
                        if debug:
                            nc.sync.dma_start(
                                dbg_h.rearrange("m p (t n) -> m p t n", t=T)
                                [m][:, :, c * NCH:(c + 1) * NCH], hm[:])

                    for i in range(CI6):
                        ps2 = b_ps.tile([128, T, NCH], F32, name="bps", tag="bps")
                        for k in range(M24):
                            lhsT = w2_sb[:, k * C + i * 128:k * C + (i + 1) * 128]
                            for hf in range(2):
                                nc.tensor.matmul(
                                    ps2[:, 2 * hf:2 * hf + 2, :], lhsT,
                                    h_tiles[k][:, 2 * hf:2 * hf + 2, :],
                                    start=(k == 0), stop=(k == M24 - 1))
                        msp = obpool.tile([128, T, NCH], BF16, name="msp", tag="msp")
                        _lif_head(nc, (ubpool, gbpool),
                                  lambda t, ps2=ps2: ps2[:, t, :],
                                  b2_sb[:, i:i + 1],
                                  lambda t, msp=msp: msp[:, t, :],
                                  VTH2)
                        xf = obpool.tile([128, T, NCH], F32, name="xf", tag="xf")
                        nc.sync.dma_start(xf[:], tok3(xT_f32[i], c))
                        nc.gpsimd.tensor_tensor(xf[:], xf[:], opb[i][:], ALU.add)
                        nc.gpsimd.tensor_tensor(xf[:], xf[:], msp[:], ALU.add)
                        nc.sync.dma_start(tok3(outT[i], c), xf[:])

    nc.compile()
    return nc


# ---------------- host-side preparation ----------------

def _fold(w, s):
    return (w * s[:, None]).astype(np.float32)


def _prep_shared(qw, qb, qs, qt, kw, kb, ks, kt, vw, vb, vs, vt,
                 pw, pb, ps, pt, f1w, f1b, f1s, f1t, f2w, f2b, f2s, f2t):
    """weights/biases/patterns shared by all cores"""
    out = {}
    # q/k: lhsT tiles [128, ci*768+o] = w'[o, 128ci+p]
    for name, w, bb, ss, tt in (("q", qw, qb, qs, qt), ("k", kw, kb, ks, kt)):
        wf = _fold(w, ss)                      # [C, C] = [out, in]
        arr = np.empty((128, CI6 * C), dtype=BF)
        for ci in range(CI6):
            arr[:, ci * C:(ci + 1) * C] = wf[:, ci * 128:(ci + 1) * 128].T.astype(BF)
        out["w" + name] = arr
        bias = (bb * ss + tt).astype(np.float32)          # [C]
        out["b" + name] = np.ascontiguousarray(bias.reshape(CI6, 128).T)  # [128, 6]
    # v: moving tiles [128, ci*768+o] = w'[o, 128ci+p]
    wf = _fold(vw, vs)
    arr = np.empty((128, CI6 * C), dtype=BF)
    for ci in range(CI6):
        arr[:, ci * C:(ci + 1) * C] = wf[:, ci * 128:(ci + 1) * 128].T.astype(BF)
    out["wv"] = arr
    bv = (vb * vs + vt).astype(np.float32)
    out["bvf"] = np.tile(bv[None, :], (128, 1)).astype(BF)
    # proj: lhsT [128, ci*768+o] = w'[o, 128ci+p]
    wf = _fold(pw, ps)
    arr = np.empty((128, CI6 * C), dtype=BF)
    for ci in range(CI6):
        arr[:, ci * C:(ci + 1) * C] = wf[:, ci * 128:(ci + 1) * 128].T.astype(BF)
    out["wp"] = arr
    bpv = (pb * ps + pt).astype(np.float32)
    out["bp"] = np.ascontiguousarray(bpv.reshape(CI6, 128).T)     # [128, 6]
    # f1: [128, ci*3072+o] = w'[o, 128ci+p]
    wf = _fold(f1w, f1s)
    arr = np.empty((128, CI6 * HD), dtype=BF)
    for ci in range(CI6):
        arr[:, ci * HD:(ci + 1) * HD] = wf[:, ci * 128:(ci + 1) * 128].T.astype(BF)
    out["w1"] = arr
    b1v = (f1b * f1s + f1t).astype(np.float32)
    out["b1"] = np.ascontiguousarray(b1v.reshape(M24, 128).T)     # [128, 24]
    # f2: [128, k*768+o] = w'[o, 128k+p]
    wf = _fold(f2w, f2s)
    arr = np.empty((128, M24 * C), dtype=BF)
    for k in range(M24):
        arr[:, k * C:(k + 1) * C] = wf[:, k * 128:(k + 1) * 128].T.astype(BF)
    out["w2"] = arr
    b2v = (f2b * f2s + f2t).astype(np.float32)
    out["b2"] = np.ascontiguousarray(b2v.reshape(CI6, 128).T)     # [128, 6]

    # attention mask / ext patterns
    cols = NCH * T
    qp = np.zeros((16, cols), dtype=BF)
    kp = np.zeros((16, cols), dtype=np.float32)
    for col in range(cols):
        j = col % NCH
        jm = j % 128
        g = jm // W
        qp[g, col] = 1.0
        jwin = jm + W
        for gg in range(16):
            kp[gg, col] = 0.0 if (W * gg <= jwin < W * gg + 2 * W) else NEG
    out["qext_pat"] = qp
    out["kext_pat"] = kp.astype(BF)
    khp = np.full((16, T * W), NEG, dtype=np.float32)
    khp[0, :] = 0.0                       # lookback valid only for group 0
    out["khalo_pat"] = khp.astype(BF)
    out["khalo_first"] = np.full((16, T * W), NEG, dtype=BF)
    out["ident"] = np.eye(128, dtype=BF)
    return out


def prep_in_maps(inputs):
    x = np.asarray(inputs["x"], dtype=np.float32)
    shared = _prep_shared(**{k: np.asarray(v, np.float32)
                             for k, v in inputs.items() if k != "x"})
    in_maps = []
    for b in range(B):
        xt = x[:, b].reshape(TOK, C)                 # [4096, 768]
        xT = np.ascontiguousarray(xt.T).reshape(CI6, 128, TOK)
        m = dict(shared)
        m["xT_f32"] = xT
        m["xT_bf"] = xT.astype(BF)
        in_maps.append(m)
    return in_maps


_NC_CACHE = {}


def get_nc(debug=False):
    if debug not in _NC_CACHE:
        _NC_CACHE[debug] = build_nc(debug)
    return _NC_CACHE[debug]


def assemble_output(results):
    out = np.empty((T, B, NSEQ, C), dtype=np.float32)
    for b in range(B):
        oT = results[b]["outT"].reshape(C, TOK)       # [768, 4096]
        out[:, b] = oT.T.reshape(T, NSEQ, C)
    return out


def kernel(**inputs):
    nc = get_nc(debug=False)
    in_maps = prep_in_maps(inputs)
    res = run_bass_kernel_spmd(nc, in_maps, list(range(B)))
    return assemble_output(res.results)



# revision 3
# speedup vs baseline: 1.7816x; 1.7816x over previous
"""Trainium2 Bass kernel v2 for nn_Block_59433757442280 (spiking local-attention block).

Data-parallel over B=8 (one batch element per core). All six GEMMs run in
fp8e4m3 with DoubleRow packing (K=256 per matmul pass). Attention computes
simT = k.T q directly (keys on partitions), softmax denominator via a ones
column folded into V, normalization via a 1x96 broadcast matmul + divide.
Spikes and attention probabilities stored fp8; LIF state bf16.
"""

import sys

for _p in ("/opt/trn_rl_repo",):
    if _p not in sys.path:
        sys.path.insert(0, _p)

import numpy as np
import ml_dtypes

import concourse.bass as bass
import concourse.tile as tile
from concourse import mybir, bacc
from concourse.bass_utils import run_bass_kernel_spmd

F32 = mybir.dt.float32
BF16 = mybir.dt.bfloat16
FP8 = mybir.dt.float8e4
AF = mybir.ActivationFunctionType
ALU = mybir.AluOpType
DR = mybir.MatmulPerfMode.DoubleRow
E4 = ml_dtypes.float8_e4m3
BF = ml_dtypes.bfloat16


# ---- custom fused LIF DVE op: u_t = gate(u_prev) + yb ----------------------
from concourse import dve_ops as _dve_ops
from concourse.dve_spec import Spec as _Spec, Src0 as _S0, Src1 as _S1, \
    C0 as _C0, C1 as _C1, Zero as _Z, select as _select, lower as _lower, \
    _has_src1 as _hs1
from concourse.dve_uop import DveOpSpec as _DveOpSpec


def _make_lif_fuse():
    if "LIF_FUSE_ANT" in _dve_ops._SUB_OPCODE_FOR_NAME:
        return next(o for o in _dve_ops.OPS if o.name == "LIF_FUSE_ANT")
    spec = _Spec(
        body=_select(_S0 < _C0, _S0 * _C1, _Z) + _S1,
        reference=lambda in0, in1, s0, s1, imm2:
            np.where(in0 < s0, in0 * s1, 0.0) + in1,
    )
    row = max(_dve_ops._SUB_OPCODE_FOR_NAME.values()) + 1
    assert row < 0x20
    shas = {}
    for ver in ("v3", "v4"):
        shas[ver] = _DveOpSpec(name="LIF_FUSE_ANT", opcode=row,
                               uops=_lower(spec, ver=ver),
                               rd1_en=_hs1(spec)).sha(ver)
    op = _dve_ops.DveOp("LIF_FUSE_ANT", spec, False, shas)
    _dve_ops.OPS.append(op)
    _dve_ops.CUSTOM_DVE_SPECS["LIF_FUSE_ANT"] = spec
    _dve_ops._SUB_OPCODE_FOR_NAME["LIF_FUSE_ANT"] = row
    return op


LIF_FUSE = _make_lif_fuse()

T, B, NSEQ, C, HD = 4, 8, 1024, 768, 3072
NH, DH, W = 8, 96, 8
TOK = T * NSEQ
SCALE = float(DH) ** -0.5
NEG = -240.0     # mask offset, fp8e4m3-representable; exp(SCALE*(qk+NEG)) < 1e-6
VTH2 = 2.0       # doubled threshold for qkv/proj/f1/f2 LIF
VTH2A = 1.0      # doubled threshold for attn LIF (vth=0.5)
NBLK = NSEQ // 128


def _lif(nc, pools, ps_of_t, bias_ap, spike, drain="act"):
    """Standard LIF over T steps on [128, W] tiles.

    ps_of_t(t): PSUM AP [128, W] of the GEMM output at step t.
    bias_ap: [128, 1] f32 SBUF AP or None (bias already in PSUM).
    spike(t, u): emit the spike op for step t from SBUF bf16 u.
    drain: engine that drains PSUM->SBUF (+bias): "act" or "pool".
    """
    upool, gpool = pools
    u_prev = None
    for t in range(T):
        y = ps_of_t(t)
        w = y.shape[-1]
        if u_prev is None:
            u = upool.tile([128, w], BF16, name="yb", tag="yb")
            if drain == "act":
                nc.scalar.activation(u[:], y, AF.Identity,
                                     bias=bias_ap[:] if bias_ap is not None
                                     else 0.0)
            else:
                nc.scalar.copy(u[:], y)
        elif drain == "act":
            yb = upool.tile([128, w], BF16, name="ybb", tag="ybb")
            nc.scalar.activation(yb[:], y, AF.Identity,
                                 bias=bias_ap[:] if bias_ap is not None else 0.0)
            u = upool.tile([128, w], BF16, name="u", tag="u")
            nc.vector._custom_dve(LIF_FUSE, out=u[:], in0=u_prev[:], in1=yb[:],
                                  s0=VTH2, s1=0.5)
        else:
            u = upool.tile([128, w], BF16, name="u", tag="u")
            nc.vector._custom_dve(LIF_FUSE, out=u[:], in0=u_prev[:], in1=y,
                                  s0=VTH2, s1=0.5)
        spike(t, u)
        u_prev = u


def build_nc(debug=False):
    nc = bacc.Bacc(None, target_bir_lowering=False, debug=False)

    # ---------------- DRAM ----------------
    xp_d = nc.dram_tensor("xp", [3, 128, 2, TOK], FP8, kind="ExternalInput")
    xf_d = nc.dram_tensor("xf", [6, 128, TOK], F32, kind="ExternalInput")
    wq_d = nc.dram_tensor("wq", [128, 2, 3, 6, 128], FP8, kind="ExternalInput")
    wk_d = nc.dram_tensor("wk", [128, 2, 3, 6, 128], FP8, kind="ExternalInput")
    wv_d = nc.dram_tensor("wv", [128, 2, 3, 3, 256], FP8, kind="ExternalInput")
    bv_d = nc.dram_tensor("bv", [1, 2, 3, 256], FP8, kind="ExternalInput")
    onesk_d = nc.dram_tensor("onesk", [1, 2, 128], FP8, kind="ExternalInput")
    wp_d = nc.dram_tensor("wp", [128, 2, 4, 6, 128], FP8, kind="ExternalInput")
    w1_d = nc.dram_tensor("w1", [128, 2, 3, 24, 128], FP8, kind="ExternalInput")
    w2_d = nc.dram_tensor("w2", [128, 2, 12, 6, 128], FP8, kind="ExternalInput")
    bq_d = nc.dram_tensor("bq", [128, 6], F32, kind="ExternalInput")
    bk_d = nc.dram_tensor("bk", [128, 6], F32, kind="ExternalInput")
    bp_d = nc.dram_tensor("bp", [128, 6], F32, kind="ExternalInput")
    b1c_d = nc.dram_tensor("b1c", [1, 24, 128], BF16, kind="ExternalInput")
    b2c_d = nc.dram_tensor("b2c", [1, 6, 128], BF16, kind="ExternalInput")
    qpat_d = nc.dram_tensor("qpat", [16, NSEQ], FP8, kind="ExternalInput")
    kmask_d = nc.dram_tensor("kmask", [16, NSEQ], FP8, kind="ExternalInput")
    ones96_d = nc.dram_tensor("ones96", [1, 96], BF16, kind="ExternalInput")
    outT = nc.dram_tensor("outT", [6, 128, TOK], F32, kind="ExternalOutput")
    if debug:
        dbg_q = nc.dram_tensor("dbg_q", [NH, 96, TOK], FP8, kind="ExternalOutput")
        dbg_v = nc.dram_tensor("dbg_v", [T, NBLK, 128, 776], FP8,
                               kind="ExternalOutput")
        dbg_oa = nc.dram_tensor("dbg_oa", [4, 128, 2, TOK], FP8,
                                kind="ExternalOutput")
        dbg_op = nc.dram_tensor("dbg_op", [6, 128, TOK], BF16,
                                kind="ExternalOutput")

    dmaq = [nc.sync, nc.sync, nc.sync, nc.gpsimd]
    qi = [0]

    def dma(dst, src):
        e = dmaq[qi[0] % 4]
        qi[0] += 1
        e.dma_start(dst, src)

    with tile.TileContext(nc) as tc:
        from contextlib import ExitStack
        with ExitStack() as top:
            pers = top.enter_context(tc.tile_pool(name="pers", bufs=1))

            xp = [pers.tile([128, 2, T, NSEQ], FP8, name=f"xp{p}", tag=f"xp{p}")
                  for p in range(3)]
            for p in range(3):
                dma(xp[p][:], xp_d[p].rearrange("p two (t n) -> p two t n", t=T))
            ones96 = pers.tile([1, 96], BF16, name="ones96", tag="ones96")
            dma(ones96[:], ones96_d[:])
            onesk = pers.tile([1, 2, 128], FP8, name="onesk", tag="onesk")
            dma(onesk[:], onesk_d[:])
            bq_sb = pers.tile([128, 6], F32, name="bq", tag="bq")
            dma(bq_sb[:], bq_d[:])
            bk_sb = pers.tile([128, 6], F32, name="bk", tag="bk")
            dma(bk_sb[:], bk_d[:])
            negv = pers.tile([128, 1], F32, name="negv", tag="negv")
            nc.vector.memset(negv[:], -VTH2A)
            negv2 = pers.tile([128, 1], F32, name="negv2", tag="negv2")
            nc.vector.memset(negv2[:], -VTH2)

            # oa spike storage: opened early so pool releases nest LIFO
            oa_cm = tc.tile_pool(name="oap", bufs=1)
            oap = oa_cm.__enter__()
            oa = [oap.tile([128, 2, T, NSEQ], FP8, name=f"oa{p}", tag=f"oa{p}")
                  for p in range(4)]
            for p in range(4):
                nc.gpsimd.memset(oa[p][96:128, :, :, :], 0.0)

            # q/k/v spike storage: lives A1..A2
            qkv_cm = tc.tile_pool(name="qkv", bufs=1)
            qkv = qkv_cm.__enter__()
            q_ext = [qkv.tile([112, T, NSEQ], FP8, name=f"qx{h}", tag=f"qx{h}")
                     for h in range(NH)]
            k_ext = [qkv.tile([112, T, NSEQ], FP8, name=f"kx{h}", tag=f"kx{h}")
                     for h in range(NH)]
            for h in range(NH):
                for t in range(T):
                    dma(q_ext[h][96:112, t, :], qpat_d[:])
                    dma(k_ext[h][96:112, t, :], kmask_d[:])
            vt = [[qkv.tile([128, 8, 97], FP8, name=f"vt{t}_{nb}", tag=f"vt{t}_{nb}")
                   for nb in range(NBLK)] for t in range(T)]
            for t in range(T):
                for nb in range(NBLK):
                    nc.vector.memset(vt[t][nb][:, :, 96:97], 1.0)

            # ================= A1a: q, k GEMM+LIF =================
            with ExitStack() as pa:
                wpool = pa.enter_context(tc.tile_pool(name="wqk", bufs=1))
                spool = pa.enter_context(tc.tile_pool(name="stmp", bufs=1))
                upool = pa.enter_context(tc.tile_pool(name="uA", bufs=4))
                gpool = pa.enter_context(tc.tile_pool(name="gA", bufs=4))
                qk_ps = pa.enter_context(
                    tc.tile_pool(name="qkps", bufs=2, space="PSUM"))

                wq_sb = wpool.tile([128, 2, 3, 6, 128], FP8, name="wq", tag="wq")
                dma(wq_sb[:], wq_d[:])
                wk_sb = wpool.tile([128, 2, 3, 6, 128], FP8, name="wk", tag="wk")
                dma(wk_sb[:], wk_d[:])
                stmp = [spool.tile([128, T, 512], FP8, name=f"st{i}", tag=f"st{i}")
                        for i in range(6)]

                for w_sb, b_sb, ext, sgn in ((wq_sb, bq_sb, q_ext, False),
                                             (wk_sb, bk_sb, k_ext, True)):
                    for w2 in range(2):
                        n0 = 512 * w2
                        for i in range(6):
                            ps = qk_ps.tile([128, T, 512], F32, name="qkp", tag="qkp")
                            for t in range(T):
                                for h2 in range(2):
                                    cc = n0 + 256 * h2
                                    for pr in range(3):
                                        nc.tensor.matmul(
                                            ps[:, t, 256 * h2:256 * h2 + 256],
                                            w_sb[:, :, pr, i, :],
                                            xp[pr][:, :, t, cc:cc + 256],
                                            start=(pr == 0), stop=(pr == 2),
                                            perf_mode=DR)
                            st = stmp[i]
                            if sgn:
                                spk = (lambda t, u, st=st: nc.scalar.activation(
                                    st[:, t, :], u[:], AF.Sign, bias=negv2[:]))
                            else:
                                spk = (lambda t, u, st=st: nc.vector.tensor_scalar(
                                    st[:, t, :], u[:], VTH2, None, ALU.is_ge))
                            _lif(nc, (upool, gpool),
                                 lambda t, ps=ps: ps[:, t, :],
                                 b_sb[:, i:i + 1], spk)
                        for h in range(NH):
                            cst = h * 96
                            i0, r0 = cst // 128, cst % 128
                            l0 = min(128 - r0, 96)
                            dma(ext[h][0:l0, :, n0:n0 + 512],
                                stmp[i0][r0:r0 + l0, :, :])
                            if l0 < 96:
                                dma(ext[h][l0:96, :, n0:n0 + 512],
                                    stmp[i0 + 1][0:96 - l0, :, :])

            # ================= A1b: v GEMM+LIF =================
            with ExitStack() as pv:
                wvpool = pv.enter_context(tc.tile_pool(name="wv", bufs=1))
                uvpool = pv.enter_context(tc.tile_pool(name="uv", bufs=1))
                gvpool = pv.enter_context(tc.tile_pool(name="gv", bufs=3))
                v_ps = pv.enter_context(
                    tc.tile_pool(name="vps", bufs=3, space="PSUM"))

                wv_sb = wvpool.tile([128, 2, 3, 3, 256], FP8, name="wv", tag="wv")
                dma(wv_sb[:], wv_d[:])
                bv_sb = wvpool.tile([1, 2, 3, 256], FP8, name="bv", tag="bv")
                dma(bv_sb[:], bv_d[:])

                uv = [uvpool.tile([128, 768], BF16, name=f"uv{nb}", tag=f"uv{nb}")
                      for nb in range(NBLK)]
                cv = [uvpool.tile([128, 768], BF16, name=f"cv{nb}", tag=f"cv{nb}")
                      for nb in range(NBLK)]
                for t in range(T):
                    for nb in range(NBLK):
                        vps = v_ps.tile([128, 768], F32, name="vpsm", tag="vpsm")
                        for ck in range(3):
                            for pr in range(3):
                                nc.tensor.matmul(
                                    vps[:, 256 * ck:256 * ck + 256],
                                    xp[pr][:, :, t, 128 * nb:128 * nb + 128],
                                    wv_sb[:, :, pr, ck, :],
                                    start=(pr == 0), stop=False, perf_mode=DR)
                            nc.tensor.matmul(
                                vps[:, 256 * ck:256 * ck + 256],
                                onesk[:], bv_sb[:, :, ck, :],
                                start=False, stop=True, perf_mode=DR)
                        u = uv[nb]
                        if t == 0:
                            nc.scalar.copy(u[:], vps[:])
                        else:
                            nc.vector._custom_dve(LIF_FUSE, out=u[:], in0=u[:],
                                                  in1=vps[:], s0=VTH2, s1=0.5)
                        nc.vector.tensor_scalar(
                            vt[t][nb][:, :, 0:96],
                            u[:].rearrange("p (h d) -> p h d", h=8),
                            VTH2, None, ALU.is_ge)

            # ================= A2: attention =================
            with ExitStack() as pb:
                hpool = pb.enter_context(tc.tile_pool(name="vhalo", bufs=1))
                apool = pb.enter_context(tc.tile_pool(name="attn", bufs=3))
                rpool = pb.enter_context(tc.tile_pool(name="rr", bufs=3))
                uopool = pb.enter_context(tc.tile_pool(name="uo", bufs=3))
                copool = pb.enter_context(tc.tile_pool(name="co", bufs=2))
                sim_ps = pb.enter_context(
                    tc.tile_pool(name="simps", bufs=2, space="PSUM"))
                halo_ps = pb.enter_context(
                    tc.tile_pool(name="halops", bufs=2, space="PSUM"))
                o_ps = pb.enter_context(
                    tc.tile_pool(name="ops", bufs=2, space="PSUM"))
                b_ps = pb.enter_context(
                    tc.tile_pool(name="bps", bufs=2, space="PSUM"))

                v_halo = [[hpool.tile([8, 8, 97], FP8, name=f"vh{t}_{b}",
                                      tag=f"vh{t}_{b}")
                           for b in range(NBLK)] for t in range(T)]
                for t in range(T):
                    for b in range(1, NBLK):
                        dma(v_halo[t][b][:], vt[t][b - 1][120:128, :, :])

                for h in range(NH):
                    co = [None, None]
                    for t in range(T):
                        for hf in range(2):
                            b0 = 4 * hf
                            n0 = 512 * hf
                            sps = sim_ps.tile([128, 512], F32, name="sps", tag="sps")
                            for bb in range(4):
                                b = b0 + bb
                                nc.tensor.matmul(
                                    sps[:, 128 * bb:128 * bb + 128],
                                    k_ext[h][0:112, t, 128 * b:128 * b + 128],
                                    q_ext[h][0:112, t, 128 * b:128 * b + 128],
                                    start=True, stop=True)
                            hbs = [b for b in range(b0, b0 + 4) if b > 0]
                            hps = halo_ps.tile([8, 8 * len(hbs)], F32,
                                               name="hps", tag="hps")
                            for j, b in enumerate(hbs):
                                nc.tensor.matmul(
                                    hps[:, 8 * j:8 * j + 8],
                                    k_ext[h][0:96, t, 128 * b - 8:128 * b],
                                    q_ext[h][0:96, t, 128 * b:128 * b + 8],
                                    start=True, stop=True)
                            atn = apool.tile([128, 512], FP8, name="atn", tag="atn")
                            nc.scalar.activation(atn[:], sps[:], AF.Exp,
                                                 scale=0.5 * SCALE)
                            atnh = apool.tile([8, 8 * len(hbs)], FP8,
                                              name="atnh", tag="atnh")
                            nc.scalar.activation(atnh[:], hps[:], AF.Exp,
                                                 scale=0.5 * SCALE)
                            ops_ = o_ps.tile([128, 512], F32, name="opsm", tag="opsm")
                            for bb in range(4):
                                b = b0 + bb
                                has_h = b > 0
                                nc.tensor.matmul(
                                    ops_[0:97, 128 * bb:128 * bb + 128],
                                    vt[t][b][:, h, :],
                                    atn[:, 128 * bb:128 * bb + 128],
                                    start=True, stop=True)
                                if has_h:
                                    j = hbs.index(b)
                                    nc.tensor.matmul(
                                        ops_[0:97, 128 * bb:128 * bb + 8],
                                        v_halo[t][b][:, h, :],
                                        atnh[:, 8 * j:8 * j + 8],
                                        start=False, stop=True,
                                        skip_group_check=True)
                            rcp = rpool.tile([1, 512], F32, name="rcp", tag="rcp")
                            nc.vector.reciprocal(rcp[:], ops_[96:97, :])
                            rrb = rpool.tile([1, 512], BF16, name="rrb", tag="rrb")
                            nc.scalar.activation(rrb[:], rcp[:],
                                                 AF.Identity, scale=2.0)
                            bps_ = b_ps.tile([96, 512], F32, name="bcp", tag="bcp")
                            nc.tensor.matmul(bps_[:], ones96[:], rrb[:],
                                             start=True, stop=True)
                            bsb = rpool.tile([96, 512], BF16, name="bsb", tag="bsb")
                            nc.scalar.copy(bsb[:], bps_[:])
                            on_ = uopool.tile([96, 512], BF16, name="on", tag="on")
                            nc.vector.tensor_tensor(on_[:], ops_[0:96, :], bsb[:],
                                                    ALU.mult)
                            if t == 0:
                                u = on_
                            else:
                                u = copool.tile([96, 512], BF16, name=f"uo{hf}",
                                                tag=f"uo{hf}")
                                nc.vector._custom_dve(LIF_FUSE, out=u[:],
                                                      in0=co[hf][:], in1=on_[:],
                                                      s0=VTH2A, s1=0.5)
                            nc.scalar.activation(
                                oa[h // 2][0:96, h % 2, t, n0:n0 + 512],
                                u[:], AF.Sign, bias=negv[0:96, :])
                            co[hf] = u
                if debug:
                    for h in range(NH):
                        dma(dbg_q.rearrange("h d (t n) -> h d t n", t=T)[h],
                            q_ext[h][0:96, :, :])
                    for t in range(T):
                        for nb in range(NBLK):
                            dma(dbg_v[t][nb],
                                vt[t][nb].rearrange("p h d -> p (h d)"))
                    for p in range(4):
                        dma(dbg_oa[p].rearrange("p two (t n) -> p two t n", t=T),
                            oa[p][:])

            qkv_cm.__exit__(None, None, None)

            # proj output spikes (pair-planes): live A3..end
            op_cm = tc.tile_pool(name="opp", bufs=1)
            opp = op_cm.__enter__()
            opP = [opp.tile([128, 2, T, NSEQ], FP8, name=f"op{i}", tag=f"op{i}")
                   for i in range(3)]

            # ================= A3: proj =================
            with ExitStack() as pc:
                wppool = pc.enter_context(tc.tile_pool(name="wp", bufs=1))
                upool = pc.enter_context(tc.tile_pool(name="uC", bufs=4))
                gpool = pc.enter_context(tc.tile_pool(name="gC", bufs=4))
                p_ps = pc.enter_context(
                    tc.tile_pool(name="pps", bufs=2, space="PSUM"))
                wp_sb = wppool.tile([128, 2, 4, 6, 128], FP8, name="wp", tag="wp")
                dma(wp_sb[:], wp_d[:])
                bp_sb = wppool.tile([128, 6], F32, name="bp", tag="bp")
                dma(bp_sb[:], bp_d[:])

                for w2 in range(2):
                    n0 = 512 * w2
                    for i in range(6):
                        ps = p_ps.tile([128, T, 512], F32, name="ppsm", tag="ppsm")
                        for t in range(T):
                            for h2 in range(2):
                                cc = n0 + 256 * h2
                                for pr in range(4):
                                    nc.tensor.matmul(
                                        ps[:, t, 256 * h2:256 * h2 + 256],
                                        wp_sb[:, :, pr, i, :],
                                        oa[pr][:, :, t, cc:cc + 256],
                                        start=(pr == 0), stop=(pr == 3),
                                        perf_mode=DR)
                        _lif(nc, (upool, gpool),
                             lambda t, ps=ps: ps[:, t, :],
                             bp_sb[:, i:i + 1],
                             lambda t, u, i=i, n0=n0: nc.vector.tensor_scalar(
                                 opP[i // 2][:, i % 2, t, n0:n0 + 512], u[:],
                                 VTH2, None, ALU.is_ge))
                if debug:
                    for i in range(6):
                        dma(dbg_op[i].rearrange("p (t n) -> p t n", t=T),
                            opP[i // 2][:, i % 2, :, :])

            # ================= B: MLP + residual =================
            with ExitStack() as pd:
                wbpool = pd.enter_context(tc.tile_pool(name="wB", bufs=1))
                x2pool = pd.enter_context(tc.tile_pool(name="x2", bufs=1))
                hpool2 = pd.enter_context(tc.tile_pool(name="hB", bufs=1))
                upool = pd.enter_context(tc.tile_pool(name="uB", bufs=3))
                gpool = pd.enter_context(tc.tile_pool(name="gB", bufs=3))
                xfpool = pd.enter_context(tc.tile_pool(name="xf", bufs=1))
                ostg = pd.enter_context(tc.tile_pool(name="ostg", bufs=1))
                mpool = pd.enter_context(tc.tile_pool(name="msp", bufs=1))
                f_ps = pd.enter_context(
                    tc.tile_pool(name="fps", bufs=2, space="PSUM"))

                w1_sb = wbpool.tile([128, 2, 3, 24, 128], FP8, name="w1", tag="w1")
                dma(w1_sb[:, :, :, 0:12, :], w1_d[:, :, :, 0:12, :])
                dma(w1_sb[:, :, :, 12:24, :], w1_d[:, :, :, 12:24, :])
                w2_sb = wbpool.tile([128, 2, 12, 6, 128], FP8, name="w2", tag="w2")
                dma(w2_sb[:, :, 0:6, :, :], w2_d[:, :, 0:6, :, :])
                dma(w2_sb[:, :, 6:12, :, :], w2_d[:, :, 6:12, :, :])
                b1c_sb = wbpool.tile([1, 24, 128], BF16, name="b1c", tag="b1c")
                dma(b1c_sb[:], b1c_d[:])
                b2c_sb = wbpool.tile([1, 6, 128], BF16, name="b2c", tag="b2c")
                dma(b2c_sb[:], b2c_d[:])
                onesr = wbpool.tile([1, 256], BF16, name="onesr", tag="onesr")
                nc.vector.memset(onesr[:], 1.0)

                for c in range(4):
                    n0 = 256 * c
                    x2p = [x2pool.tile([128, 2, T, 256], FP8, name=f"x2{p}",
                                       tag=f"x2{p}") for p in range(3)]
                    for p in range(3):
                        for pl in range(2):
                            nc.vector.tensor_tensor(
                                x2p[p][:, pl, :, :],
                                xp[p][:, pl, :, n0:n0 + 256],
                                opP[p][:, pl, :, n0:n0 + 256], ALU.add)
                    hp = [hpool2.tile([128, 2, T, 256], FP8, name=f"hp{pr}",
                                      tag=f"hp{pr}") for pr in range(12)]
                    for mp in range(12):
                        ps = f_ps.tile([128, T, 512], F32, name="fpsm", tag="fpsm")
                        for t in range(T):
                            for mm in range(2):
                                m = 2 * mp + mm
                                for pr in range(3):
                                    nc.tensor.matmul(
                                        ps[:, t, 256 * mm:256 * mm + 256],
                                        w1_sb[:, :, pr, m, :],
                                        x2p[pr][:, :, t, :],
                                        start=(pr == 0), stop=False, perf_mode=DR)
                                nc.tensor.matmul(
                                    ps[:, t, 256 * mm:256 * mm + 256],
                                    b1c_sb[:, m, :], onesr[:],
                                    start=False, stop=True)
                        if mp % 2 == 0:
                            spk = (lambda t, u, mp=mp: nc.scalar.activation(
                                hp[mp][:, :, t, :],
                                u[:].rearrange("p (two n) -> p two n", two=2),
                                AF.Sign, bias=negv2[:]))
                        else:
                            spk = (lambda t, u, mp=mp: nc.gpsimd.tensor_scalar(
                                hp[mp][:, :, t, :],
                                u[:].rearrange("p (two n) -> p two n", two=2),
                                VTH2, None, ALU.is_ge))
                        _lif(nc, (upool, gpool),
                             lambda t, ps=ps: ps[:, t, :], None, spk, drain="psum")
                    for ip in range(3):
                        ps = f_ps.tile([128, T, 512], F32, name="fpsm", tag="fpsm")
                        for t in range(T):
                            for ii in range(2):
                                i = 2 * ip + ii
                                for pr in range(12):
                                    nc.tensor.matmul(
                                        ps[:, t, 256 * ii:256 * ii + 256],
                                        w2_sb[:, :, pr, i, :],
                                        hp[pr][:, :, t, :],
                                        start=(pr == 0), stop=False, perf_mode=DR)
                                nc.tensor.matmul(
                                    ps[:, t, 256 * ii:256 * ii + 256],
                                    b2c_sb[:, i, :], onesr[:],
                                    start=False, stop=True)
                        msp = mpool.tile([128, 2, T, 256], BF16, name="msp",
                                         tag="msp")
                        _lif(nc, (upool, gpool),
                             lambda t, ps=ps: ps[:, t, :], None,
                             lambda t, u, msp=msp: nc.vector.tensor_scalar(
                                 msp[:, :, t, :],
                                 u[:].rearrange("p (two n) -> p two n", two=2),
                                 VTH2, None, ALU.is_ge),
                             drain="psum")
                        xf_sb = xfpool.tile([128, 2, T, 256], F32, name="xfs",
                                            tag="xfs")
                        for ii in range(2):
                            dma(xf_sb[:, ii, :, :],
                                xf_d[2 * ip + ii].rearrange("p (t n) -> p t n", t=T)
                                [:, :, n0:n0 + 256])
                        opm = mpool.tile([128, 2, T, 256], BF16, name="opm",
                                         tag="opm")
                        nc.vector.tensor_tensor(opm[:], opP[ip][:, :, :, n0:n0 + 256],
                                                msp[:], ALU.add)
                        ov = ostg.tile([128, 2, T, 256], F32, name="ov", tag="ov")
                        nc.vector.tensor_tensor(ov[:], xf_sb[:], opm[:], ALU.add)
                        for ii in range(2):
                            dma(outT[2 * ip + ii]
                                .rearrange("p (t n) -> p t n", t=T)
                                [:, :, n0:n0 + 256], ov[:, ii, :, :])

            op_cm.__exit__(None, None, None)
            oa_cm.__exit__(None, None, None)

    nc.compile()
    return nc


# ---------------- host-side preparation ----------------

def _fold(w, s):
    return (np.asarray(w, np.float64) * np.asarray(s, np.float64)[:, None]).astype(np.float32)


def _prep_shared(qw, qb, qs, qt, kw, kb, ks, kt, vw, vb, vs, vt,
                 pw, pb, ps, pt, f1w, f1b, f1s, f1t, f2w, f2b, f2s, f2t):
    out = {}

    def pack_lhsT(wf, npair, ntile):
        # arr[p, pl, pair, tile, oc] = wf[128*tile+oc, 256*pair+128*pl+p]
        arr = np.empty((128, 2, npair, ntile, 128), dtype=E4)
        for pr in range(npair):
            for pl in range(2):
                blk = wf[:, 256 * pr + 128 * pl:256 * pr + 128 * pl + 128]
                arr[:, pl, pr, :, :] = blk.T.reshape(128, ntile, 128)
        return arr

    out["wq"] = pack_lhsT(_fold(qw, qs), 3, 6)
    out["bq"] = np.ascontiguousarray(
        (np.asarray(qb) * np.asarray(qs) + np.asarray(qt))
        .astype(np.float32).reshape(6, 128).T)
    out["wk"] = pack_lhsT(_fold(kw, ks), 3, 6)
    out["bk"] = np.ascontiguousarray(
        (np.asarray(kb) * np.asarray(ks) + np.asarray(kt))
        .astype(np.float32).reshape(6, 128).T)

    wfv = _fold(vw, vs)
    arr = np.empty((128, 2, 3, 3, 256), dtype=E4)
    for pr in range(3):
        for pl in range(2):
            blk = wfv[:, 256 * pr + 128 * pl:256 * pr + 128 * pl + 128]
            arr[:, pl, pr, :, :] = blk.T.reshape(128, 3, 256)
    out["wv"] = arr
    bvv = (np.asarray(vb) * np.asarray(vs) + np.asarray(vt)).astype(np.float32)
    bv = np.zeros((1, 2, 3, 256), dtype=E4)
    bv[0, 0] = bvv.reshape(3, 256).astype(E4)
    out["bv"] = bv
    onesk = np.zeros((1, 2, 128), dtype=E4)
    onesk[0, 0] = 1.0
    out["onesk"] = onesk

    # proj: input is sign-encoded (+-1) spikes in padded 1024-channel space.
    # s = (sign+1)/2  ->  W @ s = (W/2) @ sign + rowsum(W)/2
    wfp = _fold(pw, ps)
    arrp = np.zeros((128, 2, 4, 6, 128), dtype=E4)
    half = (wfp * 0.5).astype(np.float32)
    for j in range(8):          # head j occupies padded block j, rows 0:96
        pr, pl = j // 2, j % 2
        blk = half[:, 96 * j:96 * j + 96]     # [768 out, 96 in]
        arrp[0:96, pl, pr, :, :] = blk.T.reshape(96, 6, 128)
    out["wp"] = arrp
    bpv = (np.asarray(pb) * np.asarray(ps) + np.asarray(pt)).astype(np.float32)
    bpv = bpv + 0.5 * wfp.sum(axis=1)
    out["bp"] = np.ascontiguousarray(bpv.reshape(6, 128).T)

    out["w1"] = pack_lhsT(_fold(f1w, f1s), 3, 24)
    out["b1c"] = ((np.asarray(f1b) * np.asarray(f1s) + np.asarray(f1t))
                  .astype(np.float32).reshape(1, 24, 128).astype(BF))

    # f2: h blocks from even m-pairs (m//2 even) are sign-encoded (+-1 via
    # Act Sign); odd m-pairs are plain {0,1}. Halve weights + add rowsum/2
    # bias for the sign blocks only.
    wf2 = _fold(f2w, f2s)
    w2_eff = wf2.copy()
    b2v = (np.asarray(f2b) * np.asarray(f2s) + np.asarray(f2t)).astype(np.float32)
    for m in range(24):
        if (m // 2) % 2 == 0:       # sign-encoded block
            blk = slice(128 * m, 128 * m + 128)
            b2v = b2v + 0.5 * wf2[:, blk].sum(axis=1)
            w2_eff[:, blk] = 0.5 * wf2[:, blk]
    out["w2"] = pack_lhsT(w2_eff, 12, 6)
    out["b2c"] = b2v.reshape(1, 6, 128).astype(BF)

    # k spikes are sign-encoded: sim' = k_hat.T q with exp scale 0.5*SCALE.
    # Pattern one-hot rows are 2.0 so mask contributions stay at full scale.
    qpat = np.zeros((16, NSEQ), dtype=E4)
    kmask = np.zeros((16, NSEQ), dtype=np.float32)
    for n in range(NSEQ):
        l = n % 128
        qpat[l // 8, n] = 2.0
        for g in range(16):
            lo = max(0, 8 * g - 8)
            hi = 8 * g + 8
            kmask[g, n] = 0.0 if (lo <= l < hi) else NEG
    out["qpat"] = qpat
    out["kmask"] = kmask.astype(E4)
    out["ones96"] = np.ones((1, 96), dtype=BF)
    return out


def prep_in_maps(inputs):
    x = np.asarray(inputs["x"], dtype=np.float32)
    shared = _prep_shared(**{k: np.asarray(v, np.float32)
                             for k, v in inputs.items() if k != "x"})
    in_maps = []
    for b in range(B):
        xt = np.ascontiguousarray(x[:, b].reshape(TOK, C).T)   # [C, TOK]
        xpair = np.empty((3, 128, 2, TOK), dtype=E4)
        for p in range(3):
            for pl in range(2):
                xpair[p, :, pl, :] = xt[256 * p + 128 * pl:
                                        256 * p + 128 * pl + 128, :].astype(E4)
        m = dict(shared)
        m["xp"] = xpair
        m["xf"] = np.ascontiguousarray(xt.reshape(6, 128, TOK))
        in_maps.append(m)
    return in_maps


_NC_CACHE = {}


def get_nc(debug=False):
    if debug not in _NC_CACHE:
        _NC_CACHE[debug] = build_nc(debug)
    return _NC_CACHE[debug]


def assemble_output(results):
    out = np.empty((T, B, NSEQ, C), dtype=np.float32)
    for b in range(B):
        oT = results[b]["outT"].reshape(C, TOK)
        out[:, b] = oT.T.reshape(T, NSEQ, C)
    return out


def kernel(**inputs):
    nc = get_nc(debug=False)
    in_maps = prep_in_maps(inputs)
    res = run_bass_kernel_spmd(nc, in_maps, list(range(B)))
    return assemble_output(res.results)


# revision 4
# speedup vs baseline: 1.9524x; 1.0959x over previous
"""Trainium2 Bass kernel v2 for nn_Block_59433757442280 (spiking local-attention block).

Data-parallel over B=8 (one batch element per core). All six GEMMs run in
fp8e4m3 with DoubleRow packing (K=256 per matmul pass). Attention computes
simT = k.T q directly (keys on partitions), softmax denominator via a ones
column folded into V, normalization via a 1x96 broadcast matmul + divide.
Spikes and attention probabilities stored fp8; LIF state bf16.
"""

import sys

for _p in ("/opt/trn_rl_repo",):
    if _p not in sys.path:
        sys.path.insert(0, _p)

import numpy as np
import ml_dtypes

import concourse.bass as bass
import concourse.tile as tile
from concourse import mybir, bacc
from concourse.bass_utils import run_bass_kernel_spmd

F32 = mybir.dt.float32
BF16 = mybir.dt.bfloat16
FP8 = mybir.dt.float8e4
AF = mybir.ActivationFunctionType
ALU = mybir.AluOpType
DR = mybir.MatmulPerfMode.DoubleRow
E4 = ml_dtypes.float8_e4m3
BF = ml_dtypes.bfloat16


# ---- custom fused LIF DVE op: u_t = gate(u_prev) + yb ----------------------
from concourse import dve_ops as _dve_ops
from concourse.dve_spec import Spec as _Spec, Src0 as _S0, Src1 as _S1, \
    C0 as _C0, C1 as _C1, Zero as _Z, select as _select, lower as _lower, \
    _has_src1 as _hs1
from concourse.dve_uop import DveOpSpec as _DveOpSpec


def _make_lif_fuse():
    if "LIF_FUSE_ANT" in _dve_ops._SUB_OPCODE_FOR_NAME:
        return next(o for o in _dve_ops.OPS if o.name == "LIF_FUSE_ANT")
    spec = _Spec(
        body=_select(_S0 < _C0, _S0 * _C1, _Z) + _S1,
        reference=lambda in0, in1, s0, s1, imm2:
            np.where(in0 < s0, in0 * s1, 0.0) + in1,
    )
    row = max(_dve_ops._SUB_OPCODE_FOR_NAME.values()) + 1
    assert row < 0x20
    shas = {}
    for ver in ("v3", "v4"):
        shas[ver] = _DveOpSpec(name="LIF_FUSE_ANT", opcode=row,
                               uops=_lower(spec, ver=ver),
                               rd1_en=_hs1(spec)).sha(ver)
    op = _dve_ops.DveOp("LIF_FUSE_ANT", spec, False, shas)
    _dve_ops.OPS.append(op)
    _dve_ops.CUSTOM_DVE_SPECS["LIF_FUSE_ANT"] = spec
    _dve_ops._SUB_OPCODE_FOR_NAME["LIF_FUSE_ANT"] = row
    return op


LIF_FUSE = _make_lif_fuse()

T, B, NSEQ, C, HD = 4, 8, 1024, 768, 3072
NH, DH, W = 8, 96, 8
TOK = T * NSEQ
SCALE = float(DH) ** -0.5
NEG = -240.0     # mask offset, fp8e4m3-representable; exp(SCALE*(qk+NEG)) < 1e-6
VTH2 = 2.0       # doubled threshold for qkv/proj/f1/f2 LIF
VTH2A = 1.0      # doubled threshold for attn LIF (vth=0.5)
NBLK = NSEQ // 128


def _lif(nc, pools, ps_of_t, bias_ap, spike, drain="act"):
    """Standard LIF over T steps on [128, W] tiles.

    ps_of_t(t): PSUM AP [128, W] of the GEMM output at step t.
    bias_ap: [128, 1] f32 SBUF AP or None (bias already in PSUM).
    spike(t, u): emit the spike op for step t from SBUF bf16 u.
    drain: engine that drains PSUM->SBUF (+bias): "act" or "pool".
    """
    upool, gpool = pools
    u_prev = None
    for t in range(T):
        y = ps_of_t(t)
        w = y.shape[-1]
        if u_prev is None:
            u = upool.tile([128, w], BF16, name="yb", tag="yb")
            if drain == "act":
                nc.scalar.activation(u[:], y, AF.Identity,
                                     bias=bias_ap[:] if bias_ap is not None
                                     else 0.0)
            else:
                nc.scalar.copy(u[:], y)
        elif drain == "act":
            yb = upool.tile([128, w], BF16, name="ybb", tag="ybb")
            nc.scalar.activation(yb[:], y, AF.Identity,
                                 bias=bias_ap[:] if bias_ap is not None else 0.0)
            u = upool.tile([128, w], BF16, name="u", tag="u")
            nc.vector._custom_dve(LIF_FUSE, out=u[:], in0=u_prev[:], in1=yb[:],
                                  s0=VTH2, s1=0.5)
        else:
            u = upool.tile([128, w], BF16, name="u", tag="u")
            nc.vector._custom_dve(LIF_FUSE, out=u[:], in0=u_prev[:], in1=y,
                                  s0=VTH2, s1=0.5)
        spike(t, u)
        u_prev = u


def build_nc(debug=False):
    nc = bacc.Bacc(None, target_bir_lowering=False, debug=False)

    # ---------------- DRAM ----------------
    xp_d = nc.dram_tensor("xp", [3, 128, 2, TOK], FP8, kind="ExternalInput")
    xf_d = nc.dram_tensor("xf", [6, 128, TOK], F32, kind="ExternalInput")
    wq_d = nc.dram_tensor("wq", [128, 2, 3, 6, 128], FP8, kind="ExternalInput")
    wk_d = nc.dram_tensor("wk", [128, 2, 3, 6, 128], FP8, kind="ExternalInput")
    wv_d = nc.dram_tensor("wv", [128, 2, 3, 3, 256], FP8, kind="ExternalInput")
    bv_d = nc.dram_tensor("bv", [1, 2, 3, 256], FP8, kind="ExternalInput")
    onesk_d = nc.dram_tensor("onesk", [1, 2, 128], FP8, kind="ExternalInput")
    wp_d = nc.dram_tensor("wp", [128, 2, 4, 6, 128], FP8, kind="ExternalInput")
    w1_d = nc.dram_tensor("w1", [128, 2, 3, 24, 128], FP8, kind="ExternalInput")
    w2_d = nc.dram_tensor("w2", [128, 2, 12, 6, 128], FP8, kind="ExternalInput")
    bq_d = nc.dram_tensor("bq", [128, 6], F32, kind="ExternalInput")
    bk_d = nc.dram_tensor("bk", [128, 6], F32, kind="ExternalInput")
    bp_d = nc.dram_tensor("bp", [128, 6], F32, kind="ExternalInput")
    b1c_d = nc.dram_tensor("b1c", [1, 24, 128], BF16, kind="ExternalInput")
    b2c_d = nc.dram_tensor("b2c", [1, 6, 128], BF16, kind="ExternalInput")
    qpat_d = nc.dram_tensor("qpat", [16, NSEQ], FP8, kind="ExternalInput")
    kmask_d = nc.dram_tensor("kmask", [16, NSEQ], FP8, kind="ExternalInput")
    ones96_d = nc.dram_tensor("ones96", [1, 96], BF16, kind="ExternalInput")
    outT = nc.dram_tensor("outT", [6, 128, TOK], F32, kind="ExternalOutput")
    if debug:
        dbg_q = nc.dram_tensor("dbg_q", [NH, 96, TOK], FP8, kind="ExternalOutput")
        dbg_v = nc.dram_tensor("dbg_v", [T, NBLK, 128, 776], FP8,
                               kind="ExternalOutput")
        dbg_oa = nc.dram_tensor("dbg_oa", [4, 128, 2, TOK], FP8,
                                kind="ExternalOutput")
        dbg_op = nc.dram_tensor("dbg_op", [6, 128, TOK], BF16,
                                kind="ExternalOutput")

    dmaq = [nc.sync, nc.sync, nc.sync, nc.gpsimd]
    qi = [0]

    def dma(dst, src):
        e = dmaq[qi[0] % 4]
        qi[0] += 1
        e.dma_start(dst, src)

    with tile.TileContext(nc) as tc:
        from contextlib import ExitStack
        with ExitStack() as top:
            pers = top.enter_context(tc.tile_pool(name="pers", bufs=1))

            xp = [pers.tile([128, 2, T, NSEQ], FP8, name=f"xp{p}", tag=f"xp{p}")
                  for p in range(3)]
            for p in range(3):
                dma(xp[p][:], xp_d[p].rearrange("p two (t n) -> p two t n", t=T))
            ones96 = pers.tile([1, 96], BF16, name="ones96", tag="ones96")
            dma(ones96[:], ones96_d[:])
            onesk = pers.tile([1, 2, 128], FP8, name="onesk", tag="onesk")
            dma(onesk[:], onesk_d[:])
            bq_sb = pers.tile([128, 6], F32, name="bq", tag="bq")
            dma(bq_sb[:], bq_d[:])
            bk_sb = pers.tile([128, 6], F32, name="bk", tag="bk")
            dma(bk_sb[:], bk_d[:])
            negv = pers.tile([128, 1], F32, name="negv", tag="negv")
            nc.vector.memset(negv[:], -VTH2A)
            negv2 = pers.tile([128, 1], F32, name="negv2", tag="negv2")
            nc.vector.memset(negv2[:], -VTH2)

            # oa spike storage: opened early so pool releases nest LIFO
            oa_cm = tc.tile_pool(name="oap", bufs=1)
            oap = oa_cm.__enter__()
            oa = [oap.tile([128, 2, T, NSEQ], FP8, name=f"oa{p}", tag=f"oa{p}")
                  for p in range(4)]
            for p in range(4):
                nc.gpsimd.memset(oa[p][96:128, :, :, :], 0.0)

            # q/k/v spike storage: lives A1..A2
            qkv_cm = tc.tile_pool(name="qkv", bufs=1)
            qkv = qkv_cm.__enter__()
            q_ext = [qkv.tile([112, T, NSEQ], FP8, name=f"qx{h}", tag=f"qx{h}")
                     for h in range(NH)]
            k_ext = [qkv.tile([112, T, NSEQ], FP8, name=f"kx{h}", tag=f"kx{h}")
                     for h in range(NH)]
            for h in range(NH):
                for t in range(T):
                    dma(q_ext[h][96:112, t, :], qpat_d[:])
                    dma(k_ext[h][96:112, t, :], kmask_d[:])
            vt = [[qkv.tile([128, 8, 97], FP8, name=f"vt{t}_{nb}", tag=f"vt{t}_{nb}")
                   for nb in range(NBLK)] for t in range(T)]
            for t in range(T):
                for nb in range(NBLK):
                    nc.vector.memset(vt[t][nb][:, :, 96:97], 1.0)

            # ================= A1a: q, k GEMM+LIF =================
            with ExitStack() as pa:
                wpool = pa.enter_context(tc.tile_pool(name="wqk", bufs=1))
                spool = pa.enter_context(tc.tile_pool(name="stmp", bufs=1))
                upool = pa.enter_context(tc.tile_pool(name="uA", bufs=4))
                gpool = pa.enter_context(tc.tile_pool(name="gA", bufs=4))
                qk_ps = pa.enter_context(
                    tc.tile_pool(name="qkps", bufs=2, space="PSUM"))

                wq_sb = wpool.tile([128, 2, 3, 6, 128], FP8, name="wq", tag="wq")
                dma(wq_sb[:], wq_d[:])
                wk_sb = wpool.tile([128, 2, 3, 6, 128], FP8, name="wk", tag="wk")
                dma(wk_sb[:], wk_d[:])
                stmp = [spool.tile([128, T, 512], FP8, name=f"st{i}", tag=f"st{i}")
                        for i in range(6)]

                for w_sb, b_sb, ext, sgn in ((wq_sb, bq_sb, q_ext, False),
                                             (wk_sb, bk_sb, k_ext, True)):
                    for w2 in range(2):
                        n0 = 512 * w2
                        for i in range(6):
                            ps = qk_ps.tile([128, T, 512], F32, name="qkp", tag="qkp")
                            for t in range(T):
                                for h2 in range(2):
                                    cc = n0 + 256 * h2
                                    for pr in range(3):
                                        nc.tensor.matmul(
                                            ps[:, t, 256 * h2:256 * h2 + 256],
                                            w_sb[:, :, pr, i, :],
                                            xp[pr][:, :, t, cc:cc + 256],
                                            start=(pr == 0), stop=(pr == 2),
                                            perf_mode=DR)
                            st = stmp[i]
                            if sgn:
                                spk = (lambda t, u, st=st: nc.scalar.activation(
                                    st[:, t, :], u[:], AF.Sign, bias=negv2[:]))
                            else:
                                spk = (lambda t, u, st=st: nc.gpsimd.tensor_scalar(
                                    st[:, t, :], u[:], VTH2, None, ALU.is_ge))
                            _lif(nc, (upool, gpool),
                                 lambda t, ps=ps: ps[:, t, :],
                                 b_sb[:, i:i + 1], spk)
                        for h in range(NH):
                            cst = h * 96
                            i0, r0 = cst // 128, cst % 128
                            l0 = min(128 - r0, 96)
                            dma(ext[h][0:l0, :, n0:n0 + 512],
                                stmp[i0][r0:r0 + l0, :, :])
                            if l0 < 96:
                                dma(ext[h][l0:96, :, n0:n0 + 512],
                                    stmp[i0 + 1][0:96 - l0, :, :])

            # ================= A1b: v GEMM+LIF =================
            with ExitStack() as pv:
                wvpool = pv.enter_context(tc.tile_pool(name="wv", bufs=1))
                uvpool = pv.enter_context(tc.tile_pool(name="uv", bufs=1))
                gvpool = pv.enter_context(tc.tile_pool(name="gv", bufs=3))
                v_ps = pv.enter_context(
                    tc.tile_pool(name="vps", bufs=3, space="PSUM"))

                wv_sb = wvpool.tile([128, 2, 3, 3, 256], FP8, name="wv", tag="wv")
                dma(wv_sb[:], wv_d[:])
                bv_sb = wvpool.tile([1, 2, 3, 256], FP8, name="bv", tag="bv")
                dma(bv_sb[:], bv_d[:])

                uv = [uvpool.tile([128, 768], BF16, name=f"uv{nb}", tag=f"uv{nb}")
                      for nb in range(NBLK)]
                cv = [uvpool.tile([128, 768], BF16, name=f"cv{nb}", tag=f"cv{nb}")
                      for nb in range(NBLK)]
                for t in range(T):
                    for nb in range(NBLK):
                        vps = v_ps.tile([128, 768], F32, name="vpsm", tag="vpsm")
                        for ck in range(3):
                            for pr in range(3):
                                nc.tensor.matmul(
                                    vps[:, 256 * ck:256 * ck + 256],
                                    xp[pr][:, :, t, 128 * nb:128 * nb + 128],
                                    wv_sb[:, :, pr, ck, :],
                                    start=(pr == 0), stop=False, perf_mode=DR)
                            nc.tensor.matmul(
                                vps[:, 256 * ck:256 * ck + 256],
                                onesk[:], bv_sb[:, :, ck, :],
                                start=False, stop=True, perf_mode=DR)
                        u = uv[nb]
                        if t == 0:
                            nc.scalar.copy(u[:], vps[:])
                        else:
                            nc.vector._custom_dve(LIF_FUSE, out=u[:], in0=u[:],
                                                  in1=vps[:], s0=VTH2, s1=0.5)
                        nc.gpsimd.tensor_scalar(
                            vt[t][nb][:, :, 0:96],
                            u[:].rearrange("p (h d) -> p h d", h=8),
                            VTH2, None, ALU.is_ge)

            # ================= A2: attention =================
            with ExitStack() as pb:
                hpool = pb.enter_context(tc.tile_pool(name="vhalo", bufs=1))
                apool = pb.enter_context(tc.tile_pool(name="attn", bufs=3))
                rpool = pb.enter_context(tc.tile_pool(name="rr", bufs=3))
                uopool = pb.enter_context(tc.tile_pool(name="uo", bufs=3))
                copool = pb.enter_context(tc.tile_pool(name="co", bufs=2))
                sim_ps = pb.enter_context(
                    tc.tile_pool(name="simps", bufs=2, space="PSUM"))
                halo_ps = pb.enter_context(
                    tc.tile_pool(name="halops", bufs=2, space="PSUM"))
                o_ps = pb.enter_context(
                    tc.tile_pool(name="ops", bufs=2, space="PSUM"))
                b_ps = pb.enter_context(
                    tc.tile_pool(name="bps", bufs=2, space="PSUM"))

                v_halo = [[hpool.tile([8, 8, 97], FP8, name=f"vh{t}_{b}",
                                      tag=f"vh{t}_{b}")
                           for b in range(NBLK)] for t in range(T)]
                for t in range(T):
                    for b in range(1, NBLK):
                        dma(v_halo[t][b][:], vt[t][b - 1][120:128, :, :])

                for h in range(NH):
                    co = [None, None]
                    for t in range(T):
                        for hf in range(2):
                            b0 = 4 * hf
                            n0 = 512 * hf
                            sps = sim_ps.tile([128, 512], F32, name="sps", tag="sps")
                            for bb in range(4):
                                b = b0 + bb
                                nc.tensor.matmul(
                                    sps[:, 128 * bb:128 * bb + 128],
                                    k_ext[h][0:112, t, 128 * b:128 * b + 128],
                                    q_ext[h][0:112, t, 128 * b:128 * b + 128],
                                    start=True, stop=True)
                            hbs = [b for b in range(b0, b0 + 4) if b > 0]
                            hps = halo_ps.tile([8, 8 * len(hbs)], F32,
                                               name="hps", tag="hps")
                            for j, b in enumerate(hbs):
                                nc.tensor.matmul(
                                    hps[:, 8 * j:8 * j + 8],
                                    k_ext[h][0:96, t, 128 * b - 8:128 * b],
                                    q_ext[h][0:96, t, 128 * b:128 * b + 8],
                                    start=True, stop=True)
                            atn = apool.tile([128, 512], FP8, name="atn", tag="atn")
                            nc.scalar.activation(atn[:], sps[:], AF.Exp,
                                                 scale=0.5 * SCALE)
                            atnh = apool.tile([8, 8 * len(hbs)], FP8,
                                              name="atnh", tag="atnh")
                            nc.scalar.activation(atnh[:], hps[:], AF.Exp,
                                                 scale=0.5 * SCALE)
                            ops_ = o_ps.tile([128, 512], F32, name="opsm", tag="opsm")
                            for bb in range(4):
                                b = b0 + bb
                                has_h = b > 0
                                nc.tensor.matmul(
                                    ops_[0:97, 128 * bb:128 * bb + 128],
                                    vt[t][b][:, h, :],
                                    atn[:, 128 * bb:128 * bb + 128],
                                    start=True, stop=True)
                                if has_h:
                                    j = hbs.index(b)
                                    nc.tensor.matmul(
                                        ops_[0:97, 128 * bb:128 * bb + 8],
                                        v_halo[t][b][:, h, :],
                                        atnh[:, 8 * j:8 * j + 8],
                                        start=False, stop=True,
                                        skip_group_check=True)
                            rcp = rpool.tile([1, 512], F32, name="rcp", tag="rcp")
                            nc.vector.reciprocal(rcp[:], ops_[96:97, :])
                            rrb = rpool.tile([1, 512], BF16, name="rrb", tag="rrb")
                            nc.gpsimd.tensor_scalar(rrb[:], rcp[:], 2.0,
                                                    None, ALU.mult)
                            bps_ = b_ps.tile([96, 512], F32, name="bcp", tag="bcp")
                            nc.tensor.matmul(bps_[:], ones96[:], rrb[:],
                                             start=True, stop=True)
                            bsb = rpool.tile([96, 512], BF16, name="bsb", tag="bsb")
                            nc.scalar.copy(bsb[:], bps_[:])
                            on_ = uopool.tile([96, 512], BF16, name="on", tag="on")
                            nc.vector.tensor_tensor(on_[:], ops_[0:96, :], bsb[:],
                                                    ALU.mult)
                            if t == 0:
                                u = on_
                            else:
                                u = copool.tile([96, 512], BF16, name=f"uo{hf}",
                                                tag=f"uo{hf}")
                                nc.vector._custom_dve(LIF_FUSE, out=u[:],
                                                      in0=co[hf][:], in1=on_[:],
                                                      s0=VTH2A, s1=0.5)
                            nc.scalar.activation(
                                oa[h // 2][0:96, h % 2, t, n0:n0 + 512],
                                u[:], AF.Sign, bias=negv[0:96, :])
                            co[hf] = u
                if debug:
                    for h in range(NH):
                        dma(dbg_q.rearrange("h d (t n) -> h d t n", t=T)[h],
                            q_ext[h][0:96, :, :])
                    for t in range(T):
                        for nb in range(NBLK):
                            dma(dbg_v[t][nb],
                                vt[t][nb].rearrange("p h d -> p (h d)"))
                    for p in range(4):
                        dma(dbg_oa[p].rearrange("p two (t n) -> p two t n", t=T),
                            oa[p][:])

            qkv_cm.__exit__(None, None, None)

            # proj output spikes (pair-planes): live A3..end
            op_cm = tc.tile_pool(name="opp", bufs=1)
            opp = op_cm.__enter__()
            opP = [opp.tile([128, 2, T, NSEQ], FP8, name=f"op{i}", tag=f"op{i}")
                   for i in range(3)]

            # ================= A3: proj =================
            with ExitStack() as pc:
                wppool = pc.enter_context(tc.tile_pool(name="wp", bufs=1))
                upool = pc.enter_context(tc.tile_pool(name="uC", bufs=4))
                gpool = pc.enter_context(tc.tile_pool(name="gC", bufs=4))
                p_ps = pc.enter_context(
                    tc.tile_pool(name="pps", bufs=2, space="PSUM"))
                wp_sb = wppool.tile([128, 2, 4, 6, 128], FP8, name="wp", tag="wp")
                dma(wp_sb[:], wp_d[:])
                bp_sb = wppool.tile([128, 6], F32, name="bp", tag="bp")
                dma(bp_sb[:], bp_d[:])

                for w2 in range(2):
                    n0 = 512 * w2
                    for i in range(6):
                        ps = p_ps.tile([128, T, 512], F32, name="ppsm", tag="ppsm")
                        for t in range(T):
                            for h2 in range(2):
                                cc = n0 + 256 * h2
                                for pr in range(4):
                                    nc.tensor.matmul(
                                        ps[:, t, 256 * h2:256 * h2 + 256],
                                        wp_sb[:, :, pr, i, :],
                                        oa[pr][:, :, t, cc:cc + 256],
                                        start=(pr == 0), stop=(pr == 3),
                                        perf_mode=DR)
                        _lif(nc, (upool, gpool),
                             lambda t, ps=ps: ps[:, t, :],
                             bp_sb[:, i:i + 1],
                             lambda t, u, i=i, n0=n0: nc.vector.tensor_scalar(
                                 opP[i // 2][:, i % 2, t, n0:n0 + 512], u[:],
                                 VTH2, None, ALU.is_ge))
                if debug:
                    for i in range(6):
                        dma(dbg_op[i].rearrange("p (t n) -> p t n", t=T),
                            opP[i // 2][:, i % 2, :, :])

            # ================= B: MLP + residual =================
            with ExitStack() as pd:
                wbpool = pd.enter_context(tc.tile_pool(name="wB", bufs=1))
                x2pool = pd.enter_context(tc.tile_pool(name="x2", bufs=1))
                hpool2 = pd.enter_context(tc.tile_pool(name="hB", bufs=1))
                upool = pd.enter_context(tc.tile_pool(name="uB", bufs=3))
                gpool = pd.enter_context(tc.tile_pool(name="gB", bufs=3))
                xfpool = pd.enter_context(tc.tile_pool(name="xf", bufs=1))
                ostg = pd.enter_context(tc.tile_pool(name="ostg", bufs=1))
                mpool = pd.enter_context(tc.tile_pool(name="msp", bufs=1))
                f_ps = pd.enter_context(
                    tc.tile_pool(name="fps", bufs=2, space="PSUM"))

                w1_sb = wbpool.tile([128, 2, 3, 24, 128], FP8, name="w1", tag="w1")
                dma(w1_sb[:, :, :, 0:12, :], w1_d[:, :, :, 0:12, :])
                dma(w1_sb[:, :, :, 12:24, :], w1_d[:, :, :, 12:24, :])
                w2_sb = wbpool.tile([128, 2, 12, 6, 128], FP8, name="w2", tag="w2")
                dma(w2_sb[:, :, 0:6, :, :], w2_d[:, :, 0:6, :, :])
                dma(w2_sb[:, :, 6:12, :, :], w2_d[:, :, 6:12, :, :])
                b1c_sb = wbpool.tile([1, 24, 128], BF16, name="b1c", tag="b1c")
                dma(b1c_sb[:], b1c_d[:])
                b2c_sb = wbpool.tile([1, 6, 128], BF16, name="b2c", tag="b2c")
                dma(b2c_sb[:], b2c_d[:])
                onesr = wbpool.tile([1, 256], BF16, name="onesr", tag="onesr")
                nc.vector.memset(onesr[:], 1.0)

                for c in range(4):
                    n0 = 256 * c
                    x2p = [x2pool.tile([128, 2, T, 256], FP8, name=f"x2{p}",
                                       tag=f"x2{p}") for p in range(3)]
                    for p in range(3):
                        for pl in range(2):
                            nc.vector.tensor_tensor(
                                x2p[p][:, pl, :, :],
                                xp[p][:, pl, :, n0:n0 + 256],
                                opP[p][:, pl, :, n0:n0 + 256], ALU.add)
                    hp = [hpool2.tile([128, 2, T, 256], FP8, name=f"hp{pr}",
                                      tag=f"hp{pr}") for pr in range(12)]
                    for mp in range(12):
                        ps = f_ps.tile([128, T, 512], F32, name="fpsm", tag="fpsm")
                        for t in range(T):
                            for mm in range(2):
                                m = 2 * mp + mm
                                for pr in range(3):
                                    nc.tensor.matmul(
                                        ps[:, t, 256 * mm:256 * mm + 256],
                                        w1_sb[:, :, pr, m, :],
                                        x2p[pr][:, :, t, :],
                                        start=(pr == 0), stop=False, perf_mode=DR)
                                nc.tensor.matmul(
                                    ps[:, t, 256 * mm:256 * mm + 256],
                                    b1c_sb[:, m, :], onesr[:],
                                    start=False, stop=True)
                        if mp % 2 == 0:
                            spk = (lambda t, u, mp=mp: nc.gpsimd.tensor_scalar(
                                hp[mp][:, :, t, :],
                                u[:].rearrange("p (two n) -> p two n", two=2),
                                VTH2, 0.5, ALU.is_ge, ALU.subtract))
                        else:
                            spk = (lambda t, u, mp=mp: nc.gpsimd.tensor_scalar(
                                hp[mp][:, :, t, :],
                                u[:].rearrange("p (two n) -> p two n", two=2),
                                VTH2, None, ALU.is_ge))
                        _lif(nc, (upool, gpool),
                             lambda t, ps=ps: ps[:, t, :], None, spk, drain="psum")
                    for ip in range(3):
                        ps = f_ps.tile([128, T, 512], F32, name="fpsm", tag="fpsm")
                        for t in range(T):
                            for ii in range(2):
                                i = 2 * ip + ii
                                for pr in range(12):
                                    nc.tensor.matmul(
                                        ps[:, t, 256 * ii:256 * ii + 256],
                                        w2_sb[:, :, pr, i, :],
                                        hp[pr][:, :, t, :],
                                        start=(pr == 0), stop=False, perf_mode=DR)
                                nc.tensor.matmul(
                                    ps[:, t, 256 * ii:256 * ii + 256],
                                    b2c_sb[:, i, :], onesr[:],
                                    start=False, stop=True)
                        msp = mpool.tile([128, 2, T, 256], BF16, name="msp",
                                         tag="msp")
                        _lif(nc, (upool, gpool),
                             lambda t, ps=ps: ps[:, t, :], None,
                             lambda t, u, msp=msp: nc.vector.tensor_scalar(
                                 msp[:, :, t, :],
                                 u[:].rearrange("p (two n) -> p two n", two=2),
                                 VTH2, None, ALU.is_ge),
                             drain="psum")
                        xf_sb = xfpool.tile([128, 2, T, 256], F32, name="xfs",
                                            tag="xfs")
                        for ii in range(2):
                            dma(xf_sb[:, ii, :, :],
                                xf_d[2 * ip + ii].rearrange("p (t n) -> p t n", t=T)
                                [:, :, n0:n0 + 256])
                        opm = mpool.tile([128, 2, T, 256], BF16, name="opm",
                                         tag="opm")
                        nc.vector.tensor_tensor(opm[:], opP[ip][:, :, :, n0:n0 + 256],
                                                msp[:], ALU.add)
                        ov = ostg.tile([128, 2, T, 256], F32, name="ov", tag="ov")
                        nc.vector.tensor_tensor(ov[:], xf_sb[:], opm[:], ALU.add)
                        for ii in range(2):
                            dma(outT[2 * ip + ii]
                                .rearrange("p (t n) -> p t n", t=T)
                                [:, :, n0:n0 + 256], ov[:, ii, :, :])

            op_cm.__exit__(None, None, None)
            oa_cm.__exit__(None, None, None)

    nc.compile()
    return nc


# ---------------- host-side preparation ----------------

def _fold(w, s):
    return (np.asarray(w, np.float64) * np.asarray(s, np.float64)[:, None]).astype(np.float32)


def _prep_shared(qw, qb, qs, qt, kw, kb, ks, kt, vw, vb, vs, vt,
                 pw, pb, ps, pt, f1w, f1b, f1s, f1t, f2w, f2b, f2s, f2t):
    out = {}

    def pack_lhsT(wf, npair, ntile):
        # arr[p, pl, pair, tile, oc] = wf[128*tile+oc, 256*pair+128*pl+p]
        arr = np.empty((128, 2, npair, ntile, 128), dtype=E4)
        for pr in range(npair):
            for pl in range(2):
                blk = wf[:, 256 * pr + 128 * pl:256 * pr + 128 * pl + 128]
                arr[:, pl, pr, :, :] = blk.T.reshape(128, ntile, 128)
        return arr

    out["wq"] = pack_lhsT(_fold(qw, qs), 3, 6)
    out["bq"] = np.ascontiguousarray(
        (np.asarray(qb) * np.asarray(qs) + np.asarray(qt))
        .astype(np.float32).reshape(6, 128).T)
    out["wk"] = pack_lhsT(_fold(kw, ks), 3, 6)
    out["bk"] = np.ascontiguousarray(
        (np.asarray(kb) * np.asarray(ks) + np.asarray(kt))
        .astype(np.float32).reshape(6, 128).T)

    wfv = _fold(vw, vs)
    arr = np.empty((128, 2, 3, 3, 256), dtype=E4)
    for pr in range(3):
        for pl in range(2):
            blk = wfv[:, 256 * pr + 128 * pl:256 * pr + 128 * pl + 128]
            arr[:, pl, pr, :, :] = blk.T.reshape(128, 3, 256)
    out["wv"] = arr
    bvv = (np.asarray(vb) * np.asarray(vs) + np.asarray(vt)).astype(np.float32)
    bv = np.zeros((1, 2, 3, 256), dtype=E4)
    bv[0, 0] = bvv.reshape(3, 256).astype(E4)
    out["bv"] = bv
    onesk = np.zeros((1, 2, 128), dtype=E4)
    onesk[0, 0] = 1.0
    out["onesk"] = onesk

    # proj: input is sign-encoded (+-1) spikes in padded 1024-channel space.
    # s = (sign+1)/2  ->  W @ s = (W/2) @ sign + rowsum(W)/2
    wfp = _fold(pw, ps)
    arrp = np.zeros((128, 2, 4, 6, 128), dtype=E4)
    half = (wfp * 0.5).astype(np.float32)
    for j in range(8):          # head j occupies padded block j, rows 0:96
        pr, pl = j // 2, j % 2
        blk = half[:, 96 * j:96 * j + 96]     # [768 out, 96 in]
        arrp[0:96, pl, pr, :, :] = blk.T.reshape(96, 6, 128)
    out["wp"] = arrp
    bpv = (np.asarray(pb) * np.asarray(ps) + np.asarray(pt)).astype(np.float32)
    bpv = bpv + 0.5 * wfp.sum(axis=1)
    out["bp"] = np.ascontiguousarray(bpv.reshape(6, 128).T)

    out["w1"] = pack_lhsT(_fold(f1w, f1s), 3, 24)
    out["b1c"] = ((np.asarray(f1b) * np.asarray(f1s) + np.asarray(f1t))
                  .astype(np.float32).reshape(1, 24, 128).astype(BF))

    # f2: h blocks from even m-pairs (m//2 even) are sign-encoded (+-1 via
    # Act Sign); odd m-pairs are plain {0,1}. Halve weights + add rowsum/2
    # bias for the sign blocks only.
    wf2 = _fold(f2w, f2s)
    w2_eff = wf2.copy()
    b2v = (np.asarray(f2b) * np.asarray(f2s) + np.asarray(f2t)).astype(np.float32)
    for m in range(24):
        if (m // 2) % 2 == 0:       # +-0.5-encoded block: s = s_hat + 0.5
            blk = slice(128 * m, 128 * m + 128)
            b2v = b2v + 0.5 * wf2[:, blk].sum(axis=1)
    out["w2"] = pack_lhsT(w2_eff, 12, 6)
    out["b2c"] = b2v.reshape(1, 6, 128).astype(BF)

    # k spikes are sign-encoded: sim' = k_hat.T q with exp scale 0.5*SCALE.
    # Pattern one-hot rows are 2.0 so mask contributions stay at full scale.
    qpat = np.zeros((16, NSEQ), dtype=E4)
    kmask = np.zeros((16, NSEQ), dtype=np.float32)
    for n in range(NSEQ):
        l = n % 128
        qpat[l // 8, n] = 2.0
        for g in range(16):
            lo = max(0, 8 * g - 8)
            hi = 8 * g + 8
            kmask[g, n] = 0.0 if (lo <= l < hi) else NEG
    out["qpat"] = qpat
    out["kmask"] = kmask.astype(E4)
    out["ones96"] = np.ones((1, 96), dtype=BF)
    return out


def prep_in_maps(inputs):
    x = np.asarray(inputs["x"], dtype=np.float32)
    shared = _prep_shared(**{k: np.asarray(v, np.float32)
                             for k, v in inputs.items() if k != "x"})
    in_maps = []
    for b in range(B):
        xt = np.ascontiguousarray(x[:, b].reshape(TOK, C).T)   # [C, TOK]
        xpair = np.empty((3, 128, 2, TOK), dtype=E4)
        for p in range(3):
            for pl in range(2):
                xpair[p, :, pl, :] = xt[256 * p + 128 * pl:
                                        256 * p + 128 * pl + 128, :].astype(E4)
        m = dict(shared)
        m["xp"] = xpair
        m["xf"] = np.ascontiguousarray(xt.reshape(6, 128, TOK))
        in_maps.append(m)
    return in_maps


_NC_CACHE = {}


def get_nc(debug=False):
    if debug not in _NC_CACHE:
        _NC_CACHE[debug] = build_nc(debug)
    return _NC_CACHE[debug]


def assemble_output(results):
    out = np.empty((T, B, NSEQ, C), dtype=np.float32)
    for b in range(B):
        oT = results[b]["outT"].reshape(C, TOK)
        out[:, b] = oT.T.reshape(T, NSEQ, C)
    return out


def kernel(**inputs):
    nc = get_nc(debug=False)
    in_maps = prep_in_maps(inputs)
    res = run_bass_kernel_spmd(nc, in_maps, list(range(B)))
    return assemble_output(res.results)


# revision 5
# speedup vs baseline: 1.9700x; 1.0090x over previous
"""Trainium2 Bass kernel v2 for nn_Block_59433757442280 (spiking local-attention block).

Data-parallel over B=8 (one batch element per core). All six GEMMs run in
fp8e4m3 with DoubleRow packing (K=256 per matmul pass). Attention computes
simT = k.T q directly (keys on partitions), softmax denominator via a ones
column folded into V, normalization via a 1x96 broadcast matmul + divide.
Spikes and attention probabilities stored fp8; LIF state bf16.
"""

import sys

for _p in ("/opt/trn_rl_repo",):
    if _p not in sys.path:
        sys.path.insert(0, _p)

import numpy as np
import ml_dtypes

import concourse.bass as bass
import concourse.tile as tile
from concourse import mybir, bacc
from concourse.bass_utils import run_bass_kernel_spmd

F32 = mybir.dt.float32
BF16 = mybir.dt.bfloat16
FP8 = mybir.dt.float8e4
AF = mybir.ActivationFunctionType
ALU = mybir.AluOpType
DR = mybir.MatmulPerfMode.DoubleRow
E4 = ml_dtypes.float8_e4m3
BF = ml_dtypes.bfloat16


# ---- custom fused LIF DVE op: u_t = gate(u_prev) + yb ----------------------
from concourse import dve_ops as _dve_ops
from concourse.dve_spec import Spec as _Spec, Src0 as _S0, Src1 as _S1, \
    C0 as _C0, C1 as _C1, Zero as _Z, select as _select, lower as _lower, \
    _has_src1 as _hs1
from concourse.dve_uop import DveOpSpec as _DveOpSpec


def _make_lif_fuse():
    if "LIF_FUSE_ANT" in _dve_ops._SUB_OPCODE_FOR_NAME:
        return next(o for o in _dve_ops.OPS if o.name == "LIF_FUSE_ANT")
    spec = _Spec(
        body=_select(_S0 < _C0, _S0 * _C1, _Z) + _S1,
        reference=lambda in0, in1, s0, s1, imm2:
            np.where(in0 < s0, in0 * s1, 0.0) + in1,
    )
    row = max(_dve_ops._SUB_OPCODE_FOR_NAME.values()) + 1
    assert row < 0x20
    shas = {}
    for ver in ("v3", "v4"):
        shas[ver] = _DveOpSpec(name="LIF_FUSE_ANT", opcode=row,
                               uops=_lower(spec, ver=ver),
                               rd1_en=_hs1(spec)).sha(ver)
    op = _dve_ops.DveOp("LIF_FUSE_ANT", spec, False, shas)
    _dve_ops.OPS.append(op)
    _dve_ops.CUSTOM_DVE_SPECS["LIF_FUSE_ANT"] = spec
    _dve_ops._SUB_OPCODE_FOR_NAME["LIF_FUSE_ANT"] = row
    return op


LIF_FUSE = _make_lif_fuse()

T, B, NSEQ, C, HD = 4, 8, 1024, 768, 3072
NH, DH, W = 8, 96, 8
TOK = T * NSEQ
SCALE = float(DH) ** -0.5
NEG = -240.0     # mask offset, fp8e4m3-representable; exp(SCALE*(qk+NEG)) < 1e-6
VTH2 = 2.0       # doubled threshold for qkv/proj/f1/f2 LIF
VTH2A = 1.0      # doubled threshold for attn LIF (vth=0.5)
NBLK = NSEQ // 128


def _lif(nc, pools, ps_of_t, bias_ap, spike, drain="act"):
    """Standard LIF over T steps on [128, W] tiles.

    ps_of_t(t): PSUM AP [128, W] of the GEMM output at step t.
    bias_ap: [128, 1] f32 SBUF AP or None (bias already in PSUM).
    spike(t, u): emit the spike op for step t from SBUF bf16 u.
    drain: engine that drains PSUM->SBUF (+bias): "act" or "pool".
    """
    upool, gpool = pools
    u_prev = None
    for t in range(T):
        y = ps_of_t(t)
        w = y.shape[-1]
        if u_prev is None:
            u = upool.tile([128, w], BF16, name="yb", tag="yb")
            if drain == "act":
                nc.scalar.activation(u[:], y, AF.Identity,
                                     bias=bias_ap[:] if bias_ap is not None
                                     else 0.0)
            else:
                nc.scalar.copy(u[:], y)
        elif drain == "act":
            yb = upool.tile([128, w], BF16, name="ybb", tag="ybb")
            nc.scalar.activation(yb[:], y, AF.Identity,
                                 bias=bias_ap[:] if bias_ap is not None else 0.0)
            u = upool.tile([128, w], BF16, name="u", tag="u")
            nc.vector._custom_dve(LIF_FUSE, out=u[:], in0=u_prev[:], in1=yb[:],
                                  s0=VTH2, s1=0.5)
        else:
            u = upool.tile([128, w], BF16, name="u", tag="u")
            nc.vector._custom_dve(LIF_FUSE, out=u[:], in0=u_prev[:], in1=y,
                                  s0=VTH2, s1=0.5)
        spike(t, u)
        u_prev = u


def build_nc(debug=False):
    nc = bacc.Bacc(None, target_bir_lowering=False, debug=False)

    # ---------------- DRAM ----------------
    xp_d = nc.dram_tensor("xp", [3, 128, 2, TOK], FP8, kind="ExternalInput")
    xf_d = nc.dram_tensor("xf", [6, 128, TOK], F32, kind="ExternalInput")
    wq_d = nc.dram_tensor("wq", [128, 2, 3, 6, 128], FP8, kind="ExternalInput")
    wk_d = nc.dram_tensor("wk", [128, 2, 3, 6, 128], FP8, kind="ExternalInput")
    wv_d = nc.dram_tensor("wv", [128, 2, 3, 3, 256], FP8, kind="ExternalInput")
    bv_d = nc.dram_tensor("bv", [1, 2, 3, 256], FP8, kind="ExternalInput")
    onesk_d = nc.dram_tensor("onesk", [1, 2, 128], FP8, kind="ExternalInput")
    wp_d = nc.dram_tensor("wp", [128, 2, 4, 6, 128], FP8, kind="ExternalInput")
    w1_d = nc.dram_tensor("w1", [128, 2, 3, 24, 128], FP8, kind="ExternalInput")
    w2_d = nc.dram_tensor("w2", [128, 2, 12, 6, 128], FP8, kind="ExternalInput")
    bq_d = nc.dram_tensor("bq", [128, 6], F32, kind="ExternalInput")
    bk_d = nc.dram_tensor("bk", [128, 6], F32, kind="ExternalInput")
    bp_d = nc.dram_tensor("bp", [128, 6], F32, kind="ExternalInput")
    b1c_d = nc.dram_tensor("b1c", [1, 24, 128], BF16, kind="ExternalInput")
    b2c_d = nc.dram_tensor("b2c", [1, 6, 128], BF16, kind="ExternalInput")
    qpat_d = nc.dram_tensor("qpat", [16, NSEQ], FP8, kind="ExternalInput")
    kmask_d = nc.dram_tensor("kmask", [16, NSEQ], FP8, kind="ExternalInput")
    ones96_d = nc.dram_tensor("ones96", [1, 96], BF16, kind="ExternalInput")
    outT = nc.dram_tensor("outT", [6, 128, TOK], F32, kind="ExternalOutput")
    if debug:
        dbg_q = nc.dram_tensor("dbg_q", [NH, 96, TOK], FP8, kind="ExternalOutput")
        dbg_v = nc.dram_tensor("dbg_v", [T, NBLK, 128, 776], FP8,
                               kind="ExternalOutput")
        dbg_oa = nc.dram_tensor("dbg_oa", [4, 128, 2, TOK], FP8,
                                kind="ExternalOutput")
        dbg_op = nc.dram_tensor("dbg_op", [6, 128, TOK], BF16,
                                kind="ExternalOutput")

    dmaq = [nc.sync, nc.sync, nc.sync, nc.gpsimd]
    qi = [0]

    def dma(dst, src):
        e = dmaq[qi[0] % 4]
        qi[0] += 1
        e.dma_start(dst, src)

    with tile.TileContext(nc) as tc:
        from contextlib import ExitStack
        with ExitStack() as top:
            pers = top.enter_context(tc.tile_pool(name="pers", bufs=1))

            xp = [pers.tile([128, 2, T, NSEQ], FP8, name=f"xp{p}", tag=f"xp{p}")
                  for p in range(3)]
            for p in range(3):
                dma(xp[p][:], xp_d[p].rearrange("p two (t n) -> p two t n", t=T))
            ones96 = pers.tile([1, 96], BF16, name="ones96", tag="ones96")
            dma(ones96[:], ones96_d[:])
            onesk = pers.tile([1, 2, 128], FP8, name="onesk", tag="onesk")
            dma(onesk[:], onesk_d[:])
            bq_sb = pers.tile([128, 6], F32, name="bq", tag="bq")
            dma(bq_sb[:], bq_d[:])
            bk_sb = pers.tile([128, 6], F32, name="bk", tag="bk")
            dma(bk_sb[:], bk_d[:])
            negv = pers.tile([128, 1], F32, name="negv", tag="negv")
            nc.vector.memset(negv[:], -VTH2A)
            negv2 = pers.tile([128, 1], F32, name="negv2", tag="negv2")
            nc.vector.memset(negv2[:], -VTH2)

            # oa spike storage: opened early so pool releases nest LIFO
            oa_cm = tc.tile_pool(name="oap", bufs=1)
            oap = oa_cm.__enter__()
            oa = [oap.tile([128, 2, T, NSEQ], FP8, name=f"oa{p}", tag=f"oa{p}")
                  for p in range(4)]
            for p in range(4):
                nc.gpsimd.memset(oa[p][96:128, :, :, :], 0.0)

            # q/k/v spike storage: lives A1..A2
            qkv_cm = tc.tile_pool(name="qkv", bufs=1)
            qkv = qkv_cm.__enter__()
            q_ext = [qkv.tile([112, T, NSEQ], FP8, name=f"qx{h}", tag=f"qx{h}")
                     for h in range(NH)]
            k_ext = [qkv.tile([112, T, NSEQ], FP8, name=f"kx{h}", tag=f"kx{h}")
                     for h in range(NH)]
            for h in range(NH):
                for t in range(T):
                    dma(q_ext[h][96:112, t, :], qpat_d[:])
                    dma(k_ext[h][96:112, t, :], kmask_d[:])
            vt = [[qkv.tile([128, 8, 97], FP8, name=f"vt{t}_{nb}", tag=f"vt{t}_{nb}")
                   for nb in range(NBLK)] for t in range(T)]
            for t in range(T):
                for nb in range(NBLK):
                    nc.vector.memset(vt[t][nb][:, :, 96:97], 1.0)

            # ================= A1a: q, k GEMM+LIF =================
            with ExitStack() as pa:
                wpool = pa.enter_context(tc.tile_pool(name="wqk", bufs=1))
                spool = pa.enter_context(tc.tile_pool(name="stmp", bufs=1))
                upool = pa.enter_context(tc.tile_pool(name="uA", bufs=4))
                gpool = pa.enter_context(tc.tile_pool(name="gA", bufs=4))
                qk_ps = pa.enter_context(
                    tc.tile_pool(name="qkps", bufs=2, space="PSUM"))

                wq_sb = wpool.tile([128, 2, 3, 6, 128], FP8, name="wq", tag="wq")
                dma(wq_sb[:], wq_d[:])
                wk_sb = wpool.tile([128, 2, 3, 6, 128], FP8, name="wk", tag="wk")
                dma(wk_sb[:], wk_d[:])
                stmp = [spool.tile([128, T, 512], FP8, name=f"st{i}", tag=f"st{i}")
                        for i in range(6)]

                for w_sb, b_sb, ext, sgn in ((wq_sb, bq_sb, q_ext, False),
                                             (wk_sb, bk_sb, k_ext, True)):
                    for w2 in range(2):
                        n0 = 512 * w2
                        for i in range(6):
                            ps = qk_ps.tile([128, T, 512], F32, name="qkp", tag="qkp")
                            for t in range(T):
                                for h2 in range(2):
                                    cc = n0 + 256 * h2
                                    for pr in range(3):
                                        nc.tensor.matmul(
                                            ps[:, t, 256 * h2:256 * h2 + 256],
                                            w_sb[:, :, pr, i, :],
                                            xp[pr][:, :, t, cc:cc + 256],
                                            start=(pr == 0), stop=(pr == 2),
                                            perf_mode=DR)
                            st = stmp[i]
                            if sgn:
                                spk = (lambda t, u, st=st: nc.scalar.activation(
                                    st[:, t, :], u[:], AF.Sign, bias=negv2[:]))
                            else:
                                spk = (lambda t, u, st=st: nc.gpsimd.tensor_scalar(
                                    st[:, t, :], u[:], VTH2, None, ALU.is_ge))
                            _lif(nc, (upool, gpool),
                                 lambda t, ps=ps: ps[:, t, :],
                                 b_sb[:, i:i + 1], spk)
                        for h in range(NH):
                            cst = h * 96
                            i0, r0 = cst // 128, cst % 128
                            l0 = min(128 - r0, 96)
                            dma(ext[h][0:l0, :, n0:n0 + 512],
                                stmp[i0][r0:r0 + l0, :, :])
                            if l0 < 96:
                                dma(ext[h][l0:96, :, n0:n0 + 512],
                                    stmp[i0 + 1][0:96 - l0, :, :])

            # ================= A1b: v GEMM+LIF =================
            with ExitStack() as pv:
                wvpool = pv.enter_context(tc.tile_pool(name="wv", bufs=1))
                uvpool = pv.enter_context(tc.tile_pool(name="uv", bufs=1))
                gvpool = pv.enter_context(tc.tile_pool(name="gv", bufs=3))
                v_ps = pv.enter_context(
                    tc.tile_pool(name="vps", bufs=3, space="PSUM"))

                wv_sb = wvpool.tile([128, 2, 3, 3, 256], FP8, name="wv", tag="wv")
                dma(wv_sb[:], wv_d[:])
                bv_sb = wvpool.tile([1, 2, 3, 256], FP8, name="bv", tag="bv")
                dma(bv_sb[:], bv_d[:])

                uv = [uvpool.tile([128, 768], BF16, name=f"uv{nb}", tag=f"uv{nb}")
                      for nb in range(NBLK)]
                cv = [uvpool.tile([128, 768], BF16, name=f"cv{nb}", tag=f"cv{nb}")
                      for nb in range(NBLK)]
                for t in range(T):
                    for nb in range(NBLK):
                        vps = v_ps.tile([128, 768], F32, name="vpsm", tag="vpsm")
                        for ck in range(3):
                            for pr in range(3):
                                nc.tensor.matmul(
                                    vps[:, 256 * ck:256 * ck + 256],
                                    xp[pr][:, :, t, 128 * nb:128 * nb + 128],
                                    wv_sb[:, :, pr, ck, :],
                                    start=(pr == 0), stop=False, perf_mode=DR)
                            nc.tensor.matmul(
                                vps[:, 256 * ck:256 * ck + 256],
                                onesk[:], bv_sb[:, :, ck, :],
                                start=False, stop=True, perf_mode=DR)
                        u = uv[nb]
                        if t == 0:
                            nc.scalar.copy(u[:], vps[:])
                        else:
                            nc.vector._custom_dve(LIF_FUSE, out=u[:], in0=u[:],
                                                  in1=vps[:], s0=VTH2, s1=0.5)
                        nc.gpsimd.tensor_scalar(
                            vt[t][nb][:, :, 0:96],
                            u[:].rearrange("p (h d) -> p h d", h=8),
                            VTH2, None, ALU.is_ge)

            # ================= A2: attention =================
            with ExitStack() as pb:
                hpool = pb.enter_context(tc.tile_pool(name="vhalo", bufs=1))
                apool = pb.enter_context(tc.tile_pool(name="attn", bufs=3))
                rpool = pb.enter_context(tc.tile_pool(name="rr", bufs=3))
                uopool = pb.enter_context(tc.tile_pool(name="uo", bufs=3))
                copool = pb.enter_context(tc.tile_pool(name="co", bufs=2))
                sim_ps = pb.enter_context(
                    tc.tile_pool(name="simps", bufs=2, space="PSUM"))
                halo_ps = pb.enter_context(
                    tc.tile_pool(name="halops", bufs=2, space="PSUM"))
                o_ps = pb.enter_context(
                    tc.tile_pool(name="ops", bufs=2, space="PSUM"))
                b_ps = pb.enter_context(
                    tc.tile_pool(name="bps", bufs=2, space="PSUM"))

                v_halo = [[hpool.tile([8, 8, 97], FP8, name=f"vh{t}_{b}",
                                      tag=f"vh{t}_{b}")
                           for b in range(NBLK)] for t in range(T)]
                for t in range(T):
                    for b in range(1, NBLK):
                        dma(v_halo[t][b][:], vt[t][b - 1][120:128, :, :])

                for h in range(NH):
                    co = [None, None]
                    for t in range(T):
                        for hf in range(2):
                            b0 = 4 * hf
                            n0 = 512 * hf
                            sps = sim_ps.tile([128, 512], F32, name="sps", tag="sps")
                            for bb in range(4):
                                b = b0 + bb
                                nc.tensor.matmul(
                                    sps[:, 128 * bb:128 * bb + 128],
                                    k_ext[h][0:112, t, 128 * b:128 * b + 128],
                                    q_ext[h][0:112, t, 128 * b:128 * b + 128],
                                    start=True, stop=True)
                            hbs = [b for b in range(b0, b0 + 4) if b > 0]
                            hps = halo_ps.tile([8, 8 * len(hbs)], F32,
                                               name="hps", tag="hps")
                            for j, b in enumerate(hbs):
                                nc.tensor.matmul(
                                    hps[:, 8 * j:8 * j + 8],
                                    k_ext[h][0:96, t, 128 * b - 8:128 * b],
                                    q_ext[h][0:96, t, 128 * b:128 * b + 8],
                                    start=True, stop=True)
                            atn = apool.tile([128, 512], FP8, name="atn", tag="atn")
                            nc.scalar.activation(atn[:], sps[:], AF.Exp,
                                                 scale=0.5 * SCALE)
                            atnh = apool.tile([8, 8 * len(hbs)], FP8,
                                              name="atnh", tag="atnh")
                            nc.scalar.activation(atnh[:], hps[:], AF.Exp,
                                                 scale=0.5 * SCALE)
                            ops_ = o_ps.tile([128, 512], F32, name="opsm", tag="opsm")
                            for bb in range(4):
                                b = b0 + bb
                                has_h = b > 0
                                nc.tensor.matmul(
                                    ops_[0:97, 128 * bb:128 * bb + 128],
                                    vt[t][b][:, h, :],
                                    atn[:, 128 * bb:128 * bb + 128],
                                    start=True, stop=True)
                                if has_h:
                                    j = hbs.index(b)
                                    nc.tensor.matmul(
                                        ops_[0:97, 128 * bb:128 * bb + 8],
                                        v_halo[t][b][:, h, :],
                                        atnh[:, 8 * j:8 * j + 8],
                                        start=False, stop=True,
                                        skip_group_check=True)
                            rcp = rpool.tile([1, 512], F32, name="rcp", tag="rcp")
                            nc.vector.reciprocal(rcp[:], ops_[96:97, :])
                            rrb = rpool.tile([1, 512], BF16, name="rrb", tag="rrb")
                            nc.gpsimd.tensor_scalar(rrb[:], rcp[:], 2.0,
                                                    None, ALU.mult)
                            bps_ = b_ps.tile([96, 512], F32, name="bcp", tag="bcp")
                            nc.tensor.matmul(bps_[:], ones96[:], rrb[:],
                                             start=True, stop=True)
                            bsb = rpool.tile([96, 512], BF16, name="bsb", tag="bsb")
                            nc.scalar.copy(bsb[:], bps_[:])
                            on_ = uopool.tile([96, 512], BF16, name="on", tag="on")
                            nc.vector.tensor_tensor(on_[:], ops_[0:96, :], bsb[:],
                                                    ALU.mult)
                            if t == 0:
                                u = on_
                            else:
                                u = copool.tile([96, 512], BF16, name=f"uo{hf}",
                                                tag=f"uo{hf}")
                                nc.vector._custom_dve(LIF_FUSE, out=u[:],
                                                      in0=co[hf][:], in1=on_[:],
                                                      s0=VTH2A, s1=0.5)
                            nc.scalar.activation(
                                oa[h // 2][0:96, h % 2, t, n0:n0 + 512],
                                u[:], AF.Sign, bias=negv[0:96, :])
                            co[hf] = u
                if debug:
                    for h in range(NH):
                        dma(dbg_q.rearrange("h d (t n) -> h d t n", t=T)[h],
                            q_ext[h][0:96, :, :])
                    for t in range(T):
                        for nb in range(NBLK):
                            dma(dbg_v[t][nb],
                                vt[t][nb].rearrange("p h d -> p (h d)"))
                    for p in range(4):
                        dma(dbg_oa[p].rearrange("p two (t n) -> p two t n", t=T),
                            oa[p][:])

            qkv_cm.__exit__(None, None, None)

            # proj output spikes (pair-planes): live A3..end
            op_cm = tc.tile_pool(name="opp", bufs=1)
            opp = op_cm.__enter__()
            opP = [opp.tile([128, 2, T, NSEQ], FP8, name=f"op{i}", tag=f"op{i}")
                   for i in range(3)]

            # ================= A3: proj =================
            with ExitStack() as pc:
                wppool = pc.enter_context(tc.tile_pool(name="wp", bufs=1))
                upool = pc.enter_context(tc.tile_pool(name="uC", bufs=4))
                gpool = pc.enter_context(tc.tile_pool(name="gC", bufs=4))
                p_ps = pc.enter_context(
                    tc.tile_pool(name="pps", bufs=2, space="PSUM"))
                wp_sb = wppool.tile([128, 2, 4, 6, 128], FP8, name="wp", tag="wp")
                dma(wp_sb[:], wp_d[:])
                bp_sb = wppool.tile([128, 6], F32, name="bp", tag="bp")
                dma(bp_sb[:], bp_d[:])

                for w2 in range(2):
                    n0 = 512 * w2
                    for i in range(6):
                        ps = p_ps.tile([128, T, 512], F32, name="ppsm", tag="ppsm")
                        for t in range(T):
                            for h2 in range(2):
                                cc = n0 + 256 * h2
                                for pr in range(4):
                                    nc.tensor.matmul(
                                        ps[:, t, 256 * h2:256 * h2 + 256],
                                        wp_sb[:, :, pr, i, :],
                                        oa[pr][:, :, t, cc:cc + 256],
                                        start=(pr == 0), stop=(pr == 3),
                                        perf_mode=DR)
                        _lif(nc, (upool, gpool),
                             lambda t, ps=ps: ps[:, t, :],
                             bp_sb[:, i:i + 1],
                             lambda t, u, i=i, n0=n0: nc.vector.tensor_scalar(
                                 opP[i // 2][:, i % 2, t, n0:n0 + 512], u[:],
                                 VTH2, None, ALU.is_ge))
                if debug:
                    for i in range(6):
                        dma(dbg_op[i].rearrange("p (t n) -> p t n", t=T),
                            opP[i // 2][:, i % 2, :, :])

            # ================= B: MLP + residual =================
            with ExitStack() as pd:
                wbpool = pd.enter_context(tc.tile_pool(name="wB", bufs=1))
                x2pool = pd.enter_context(tc.tile_pool(name="x2", bufs=1))
                hpool2 = pd.enter_context(tc.tile_pool(name="hB", bufs=1))
                upool = pd.enter_context(tc.tile_pool(name="uB", bufs=3))
                gpool = pd.enter_context(tc.tile_pool(name="gB", bufs=3))
                xfpool = pd.enter_context(tc.tile_pool(name="xf", bufs=1))
                ostg = pd.enter_context(tc.tile_pool(name="ostg", bufs=1))
                mpool = pd.enter_context(tc.tile_pool(name="msp", bufs=1))
                f_ps = pd.enter_context(
                    tc.tile_pool(name="fps", bufs=2, space="PSUM"))

                w1_sb = wbpool.tile([128, 2, 3, 24, 128], FP8, name="w1", tag="w1")
                dma(w1_sb[:, :, :, 0:12, :], w1_d[:, :, :, 0:12, :])
                dma(w1_sb[:, :, :, 12:24, :], w1_d[:, :, :, 12:24, :])
                w2_sb = wbpool.tile([128, 2, 12, 6, 128], FP8, name="w2", tag="w2")
                dma(w2_sb[:, :, 0:6, :, :], w2_d[:, :, 0:6, :, :])
                dma(w2_sb[:, :, 6:12, :, :], w2_d[:, :, 6:12, :, :])
                b1c_sb = wbpool.tile([1, 24, 128], BF16, name="b1c", tag="b1c")
                dma(b1c_sb[:], b1c_d[:])
                b2c_sb = wbpool.tile([1, 6, 128], BF16, name="b2c", tag="b2c")
                dma(b2c_sb[:], b2c_d[:])
                onesr = wbpool.tile([1, 256], BF16, name="onesr", tag="onesr")
                nc.vector.memset(onesr[:], 1.0)

                for c in range(4):
                    n0 = 256 * c
                    x2p = [x2pool.tile([128, 2, T, 256], FP8, name=f"x2{p}",
                                       tag=f"x2{p}") for p in range(3)]
                    for p in range(3):
                        for pl in range(2):
                            nc.gpsimd.tensor_tensor(
                                x2p[p][:, pl, :, :],
                                xp[p][:, pl, :, n0:n0 + 256],
                                opP[p][:, pl, :, n0:n0 + 256], ALU.add)
                    hp = [hpool2.tile([128, 2, T, 256], FP8, name=f"hp{pr}",
                                      tag=f"hp{pr}") for pr in range(12)]
                    for mp in range(12):
                        ps = f_ps.tile([128, T, 512], F32, name="fpsm", tag="fpsm")
                        for t in range(T):
                            for mm in range(2):
                                m = 2 * mp + mm
                                for pr in range(3):
                                    nc.tensor.matmul(
                                        ps[:, t, 256 * mm:256 * mm + 256],
                                        w1_sb[:, :, pr, m, :],
                                        x2p[pr][:, :, t, :],
                                        start=(pr == 0), stop=False, perf_mode=DR)
                                nc.tensor.matmul(
                                    ps[:, t, 256 * mm:256 * mm + 256],
                                    b1c_sb[:, m, :], onesr[:],
                                    start=False, stop=True)
                        if mp % 2 == 0:
                            spk = (lambda t, u, mp=mp: nc.gpsimd.tensor_scalar(
                                hp[mp][:, :, t, :],
                                u[:].rearrange("p (two n) -> p two n", two=2),
                                VTH2, 0.5, ALU.is_ge, ALU.subtract))
                        else:
                            spk = (lambda t, u, mp=mp: nc.gpsimd.tensor_scalar(
                                hp[mp][:, :, t, :],
                                u[:].rearrange("p (two n) -> p two n", two=2),
                                VTH2, None, ALU.is_ge))
                        _lif(nc, (upool, gpool),
                             lambda t, ps=ps: ps[:, t, :], None, spk, drain="psum")
                    for ip in range(3):
                        ps = f_ps.tile([128, T, 512], F32, name="fpsm", tag="fpsm")
                        for t in range(T):
                            for ii in range(2):
                                i = 2 * ip + ii
                                for pr in range(12):
                                    nc.tensor.matmul(
                                        ps[:, t, 256 * ii:256 * ii + 256],
                                        w2_sb[:, :, pr, i, :],
                                        hp[pr][:, :, t, :],
                                        start=(pr == 0), stop=False, perf_mode=DR)
                                nc.tensor.matmul(
                                    ps[:, t, 256 * ii:256 * ii + 256],
                                    b2c_sb[:, i, :], onesr[:],
                                    start=False, stop=True)
                        msp = mpool.tile([128, 2, T, 256], BF16, name="msp",
                                         tag="msp")
                        _lif(nc, (upool, gpool),
                             lambda t, ps=ps: ps[:, t, :], None,
                             lambda t, u, msp=msp: nc.vector.tensor_scalar(
                                 msp[:, :, t, :],
                                 u[:].rearrange("p (two n) -> p two n", two=2),
                                 VTH2, None, ALU.is_ge),
                             drain="psum")
                        xf_sb = xfpool.tile([128, 2, T, 256], F32, name="xfs",
                                            tag="xfs")
                        for ii in range(2):
                            dma(xf_sb[:, ii, :, :],
                                xf_d[2 * ip + ii].rearrange("p (t n) -> p t n", t=T)
                                [:, :, n0:n0 + 256])
                        opm = mpool.tile([128, 2, T, 256], BF16, name="opm",
                                         tag="opm")
                        nc.vector.tensor_tensor(opm[:], opP[ip][:, :, :, n0:n0 + 256],
                                                msp[:], ALU.add)
                        ov = ostg.tile([128, 2, T, 256], F32, name="ov", tag="ov")
                        nc.vector.tensor_tensor(ov[:], xf_sb[:], opm[:], ALU.add)
                        for ii in range(2):
                            dma(outT[2 * ip + ii]
                                .rearrange("p (t n) -> p t n", t=T)
                                [:, :, n0:n0 + 256], ov[:, ii, :, :])

            op_cm.__exit__(None, None, None)
            oa_cm.__exit__(None, None, None)

    nc.compile()
    return nc


# ---------------- host-side preparation ----------------

def _fold(w, s):
    return (np.asarray(w, np.float64) * np.asarray(s, np.float64)[:, None]).astype(np.float32)


def _prep_shared(qw, qb, qs, qt, kw, kb, ks, kt, vw, vb, vs, vt,
                 pw, pb, ps, pt, f1w, f1b, f1s, f1t, f2w, f2b, f2s, f2t):
    out = {}

    def pack_lhsT(wf, npair, ntile):
        # arr[p, pl, pair, tile, oc] = wf[128*tile+oc, 256*pair+128*pl+p]
        arr = np.empty((128, 2, npair, ntile, 128), dtype=E4)
        for pr in range(npair):
            for pl in range(2):
                blk = wf[:, 256 * pr + 128 * pl:256 * pr + 128 * pl + 128]
                arr[:, pl, pr, :, :] = blk.T.reshape(128, ntile, 128)
        return arr

    out["wq"] = pack_lhsT(_fold(qw, qs), 3, 6)
    out["bq"] = np.ascontiguousarray(
        (np.asarray(qb) * np.asarray(qs) + np.asarray(qt))
        .astype(np.float32).reshape(6, 128).T)
    out["wk"] = pack_lhsT(_fold(kw, ks), 3, 6)
    out["bk"] = np.ascontiguousarray(
        (np.asarray(kb) * np.asarray(ks) + np.asarray(kt))
        .astype(np.float32).reshape(6, 128).T)

    wfv = _fold(vw, vs)
    arr = np.empty((128, 2, 3, 3, 256), dtype=E4)
    for pr in range(3):
        for pl in range(2):
            blk = wfv[:, 256 * pr + 128 * pl:256 * pr + 128 * pl + 128]
            arr[:, pl, pr, :, :] = blk.T.reshape(128, 3, 256)
    out["wv"] = arr
    bvv = (np.asarray(vb) * np.asarray(vs) + np.asarray(vt)).astype(np.float32)
    bv = np.zeros((1, 2, 3, 256), dtype=E4)
    bv[0, 0] = bvv.reshape(3, 256).astype(E4)
    out["bv"] = bv
    onesk = np.zeros((1, 2, 128), dtype=E4)
    onesk[0, 0] = 1.0
    out["onesk"] = onesk

    # proj: input is sign-encoded (+-1) spikes in padded 1024-channel space.
    # s = (sign+1)/2  ->  W @ s = (W/2) @ sign + rowsum(W)/2
    wfp = _fold(pw, ps)
    arrp = np.zeros((128, 2, 4, 6, 128), dtype=E4)
    half = (wfp * 0.5).astype(np.float32)
    for j in range(8):          # head j occupies padded block j, rows 0:96
        pr, pl = j // 2, j % 2
        blk = half[:, 96 * j:96 * j + 96]     # [768 out, 96 in]
        arrp[0:96, pl, pr, :, :] = blk.T.reshape(96, 6, 128)
    out["wp"] = arrp
    bpv = (np.asarray(pb) * np.asarray(ps) + np.asarray(pt)).astype(np.float32)
    bpv = bpv + 0.5 * wfp.sum(axis=1)
    out["bp"] = np.ascontiguousarray(bpv.reshape(6, 128).T)

    out["w1"] = pack_lhsT(_fold(f1w, f1s), 3, 24)
    out["b1c"] = ((np.asarray(f1b) * np.asarray(f1s) + np.asarray(f1t))
                  .astype(np.float32).reshape(1, 24, 128).astype(BF))

    # f2: h blocks from even m-pairs (m//2 even) are sign-encoded (+-1 via
    # Act Sign); odd m-pairs are plain {0,1}. Halve weights + add rowsum/2
    # bias for the sign blocks only.
    wf2 = _fold(f2w, f2s)
    w2_eff = wf2.copy()
    b2v = (np.asarray(f2b) * np.asarray(f2s) + np.asarray(f2t)).astype(np.float32)
    for m in range(24):
        if (m // 2) % 2 == 0:       # +-0.5-encoded block: s = s_hat + 0.5
            blk = slice(128 * m, 128 * m + 128)
            b2v = b2v + 0.5 * wf2[:, blk].sum(axis=1)
    out["w2"] = pack_lhsT(w2_eff, 12, 6)
    out["b2c"] = b2v.reshape(1, 6, 128).astype(BF)

    # k spikes are sign-encoded: sim' = k_hat.T q with exp scale 0.5*SCALE.
    # Pattern one-hot rows are 2.0 so mask contributions stay at full scale.
    qpat = np.zeros((16, NSEQ), dtype=E4)
    kmask = np.zeros((16, NSEQ), dtype=np.float32)
    for n in range(NSEQ):
        l = n % 128
        qpat[l // 8, n] = 2.0
        for g in range(16):
            lo = max(0, 8 * g - 8)
            hi = 8 * g + 8
            kmask[g, n] = 0.0 if (lo <= l < hi) else NEG
    out["qpat"] = qpat
    out["kmask"] = kmask.astype(E4)
    out["ones96"] = np.ones((1, 96), dtype=BF)
    return out


def prep_in_maps(inputs):
    x = np.asarray(inputs["x"], dtype=np.float32)
    shared = _prep_shared(**{k: np.asarray(v, np.float32)
                             for k, v in inputs.items() if k != "x"})
    in_maps = []
    for b in range(B):
        xt = np.ascontiguousarray(x[:, b].reshape(TOK, C).T)   # [C, TOK]
        xpair = np.empty((3, 128, 2, TOK), dtype=E4)
        for p in range(3):
            for pl in range(2):
                xpair[p, :, pl, :] = xt[256 * p + 128 * pl:
                                        256 * p + 128 * pl + 128, :].astype(E4)
        m = dict(shared)
        m["xp"] = xpair
        m["xf"] = np.ascontiguousarray(xt.reshape(6, 128, TOK))
        in_maps.append(m)
    return in_maps


_NC_CACHE = {}


def get_nc(debug=False):
    if debug not in _NC_CACHE:
        _NC_CACHE[debug] = build_nc(debug)
    return _NC_CACHE[debug]


def assemble_output(results):
    out = np.empty((T, B, NSEQ, C), dtype=np.float32)
    for b in range(B):
        oT = results[b]["outT"].reshape(C, TOK)
        out[:, b] = oT.T.reshape(T, NSEQ, C)
    return out


def kernel(**inputs):
    nc = get_nc(debug=False)
    in_maps = prep_in_maps(inputs)
    res = run_bass_kernel_spmd(nc, in_maps, list(range(B)))
    return assemble_output(res.results)
